# revision 29
# baseline (speedup 1.0000x reference)
"""Trainium2 Bass kernel for AdaptiveSemiseparableLayer (v3).

v3: the whole gate MLP (H, G, and the transposed-gate recompute GT) runs
in fp8 e4m3 with MatmulPerfMode.DoubleRow — each matmul contracts 256
inputs in the time a bf16 matmul contracts 128, halving the PE cycles of
those phases (384 -> 192 matmul-equivalents). x/Wg1/Wg2 are quantized
host-side with static power-of-2 scales (x*2^5, W*2^10); the rescale
rides the existing evacuation ops (DVE tensor_scalar for relu, ACT
activation(scale=...) for the sigmoids). GT reuses the same quantized
h8/wg28 tiles as G, so gtt == gn^T exactly and the fp8 error is NOT paid
twice. Measured end-to-end rel err ~1.2e-2 (gate 2e-2). U/V/UV stay bf16:
fp8 there measures 3.7e-2..6.5e-2 — over the gate.

Reference computation (B=4, L=2048, D=R=2048, DH=512):
    t_out = depthwise_conv1d(x, conv_w, k=3) + conv_b
    u = x @ Wu.T + bu ; v = x @ Wv.T + bv
    gates = sigmoid(relu(x @ Wg1.T + bg1) @ Wg2.T + bg2)
    cs = cumsum(u * gates, axis=seq)
    out = t_out + (cs * (v * gates)) @ Wu.T

Sharding: sequence-parallel; 8192 tokens -> 8 contiguous 1024-token blocks
(one per core; each block lies inside one batch row). The only cross-core
dependency is the cumsum carry: odd core c needs core c-1's column sums.
That is a pairwise AllReduce(add) over (R,) = 8KB; each core then forms
carry = (pair_sum - own_sum) * is_odd on GPSIMD.

Changes vs the original kernel (637 us -> ~465 us):
  - two-level cumsum: 8 independent 128-wide triangular matmuls per r-tile
    (8 MMs, 1024 PE cycles) instead of the 12-MM trapezoid; the 8-block
    prefix is recombined per 128-token window on DVE via a scan +
    scalar_tensor_tensor (cs + P) * vg.
  - block sums come from the intra-cumsum PSUM's last columns (DVE strided
    gather + reduce) -- the 128 free-dim-1 block-sum matmuls are gone.
  - the carry selection matmuls (mask MMs) are gone (pairwise AllReduce
    over core pairs + (pair_sum - own_sum) * is_odd).
  - all bias matmuls are gone: bg1 rides a DVE tensor_scalar (add,max),
    bg2(GT) rides the ACT activation bias, bv rides a
    scalar_tensor_tensor, bu/bg2(G-natural) have a DVE broadcast-add
    fallback variant only when nonzero (they are zero here).
  - the tile scheduler reorders by dependency, not emission order, so all
    carry-dependent DVE work (and the x-only conv ct chains) is pinned via
    dummy 1-element WAW copies: the carry chain behind vgt(GATE), each
    kd's ct chain behind gos[kd]. Without these the DVE FIFO stalls on the
    ~35us AllReduce latency (or runs 32 ct chains before phase H's relus).
  - H is k-outer across all 8 PSUM banks so the 16 x chunks are consumed
    at DMA arrival pace; Wg1/Wg2 preloaded whole on the ACT DMA queue.
  - gn/intra and ug/gos share SBUF pools (phase-disjoint reuse); the
    final output tiles drain over three DMA queues.

Layouts (per core, T=1024 local tokens):
    xT    [D, T+2]  bf16   transposed shard with conv halo columns
    h     [dh-part, t]     (4 tiles)
    gn    [t-part, r]      (8 tiles)   natural gates (for ug)
    ug    [t-part, r]      bf16, lhsT of the intra cumsum matmuls
    intra [r-part, t]      bf16, block-local inclusive cumsum
    gtt/vgt [r-part, t]    transposed gates / gated v
    got   [r-part, t]      (cs_local + prefix + carry) * vg
    outT  [d-part, t]      final, conv fused in epilogue
"""

import numpy as np
import ml_dtypes
from contextlib import ExitStack

import concourse.bass as bass
import concourse.mybir as mybir
import concourse.tile as tile
from concourse.bass_utils import run_bass_kernel_spmd

P = 128
B, L, D = 4, 2048, 2048
R, DH = 2048, 512
NCORES = 8
T = (B * L) // NCORES          # 1024 tokens per core
TH = T + 2                     # with conv halo
ND, NR, NDH, NT = D // P, R // P, DH // P, T // P
TC = 512                       # matmul free-dim chunk (one PSUM bank of f32)
NTC = T // TC                  # 2
NRC = R // TC                  # 4
BF = mybir.dt.bfloat16
F8 = mybir.dt.float8e4
F32 = mybir.dt.float32
AF = mybir.ActivationFunctionType
ALU = mybir.AluOpType
DR = mybir.MatmulPerfMode.DoubleRow

# fp8 static power-of-2 scales for the gate MLP (x, Wg1/Wg2 quantized to
# TRN e4m3, max-normal 240): psh/psg land at 2^(SX+SW); the rescale rides
# the evacuation op (DVE mult / ACT activation scale).
SX = 5     # x, h are scaled by 2^SX
SW = 10    # Wg1, Wg2 scaled by 2^SW

import os
DG = int(os.environ.get("K_DG", "12"))    # got lag behind gtv emission


def _emit(nc, io, zu, zg2, zbg1):
    """zu/zg2/zbg1: True when bu / bg2 / bg1 are all-zero (skip bias work)."""
    ctx = ExitStack()
    tc = io["tc"]
    pool = lambda name, bufs, **kw: ctx.enter_context(
        tc.tile_pool(name=name, bufs=bufs, **kw)
    )
    const = pool("const", 1)
    xpool = pool("xpool", 1)
    hpool = pool("hpool", 1)       # single fp8 h tile [P, NDH, T]
    gnp = pool("gnp", NT)          # gn tiles; reused for intra pairs
    ugp = pool("ugp", 2 * NT)      # ug tiles [128,1024]; reused for gos
    pbp = pool("pbp", NR)          # Pb [128, 9] f32 per rk
    ptp = pool("ptp", 3)           # Ptot transient
    gtp = pool("gtp", 2)
    vgp = pool("vgp", DG + 2)
    wutp = pool("wutp", 6)
    wvtp = pool("wvtp", 8)
    wu2p = pool("wu2p", 5)
    outp = pool("outp", 2)
    ctp = pool("ctp", 4)
    psum = pool("psum", 8, space="PSUM")
    dram = pool("dram", 1, space="DRAM")

    xT, WuT = (io[k] for k in ["xT", "WuT"])
    WvPre, Wu2Pre = io["WvPre"], io["Wu2Pre"]
    x8T, Wg18, Wg28 = (io[k] for k in ["x8T", "Wg18", "Wg28"])
    outT = io["outT"]

    # ---- critical-path loads first: fp8 Wg1 + fp8 x shard (phase H inputs).
    # All weight/const layouts are pre-shuffled HOST-side so every DMA here
    # is one contiguous run per partition (~128 descriptors): the sequencer
    # DIRECT2D launch cost stays ~0.6us instead of 2-4us, which matters
    # because launches serialize on the issuing engine's queue (the ACT
    # queue must be clear before the phase-G sigmoids can run).
    wg18 = const.tile([P, ND, DH], F8)
    x8 = xpool.tile([P, ND, T], F8, name="x8", tag="x8")
    xtile = xpool.tile([P, ND, TH], BF, name="xt", tag="xt")
    # x8 even chunks on sync, odd on gpsimd: each H pair (2 chunks) arrives
    # at two-queue rate, ~2x the pair consumption rate of the DR matmuls.
    nc.scalar.dma_start(out=wg18[:, 0:2, :], in_=Wg18[:, 0 : 2 * DH])
    for k in (0, 1):
        eng = nc.sync if k % 2 == 0 else nc.gpsimd
        eng.dma_start(out=x8[:, k, :], in_=x8T[k * P : (k + 1) * P, :])
    if not zbg1:
        bg1sc = const.tile([P, NDH], F32)
        nc.scalar.dma_start(out=bg1sc, in_=io["bg1s_col"][:, :])
    nc.scalar.dma_start(out=wg18[:, 2:8, :], in_=Wg18[:, 2 * DH : 8 * DH])
    nc.scalar.dma_start(out=wg18[:, 8:16, :], in_=Wg18[:, 8 * DH : 16 * DH])
    for k in range(2, 10):
        eng = nc.sync if k % 2 == 0 else nc.gpsimd
        eng.dma_start(out=x8[:, k, :], in_=x8T[k * P : (k + 1) * P, :])
    # fp8 Wg2 next on scalar (needed at phase G, ~20us in); after this the
    # ACT queue is free for the sigmoids.
    wg28 = const.tile([P, NDH, R], F8)
    nc.scalar.dma_start(out=wg28, in_=Wg28[:, :])
    # x8 tail chunks ride the then-idle scalar queue: 3-queue arrival keeps
    # the last H pairs ahead of the PE.
    for k in range(10, ND):
        eng = (nc.sync, nc.gpsimd, nc.scalar)[k % 3]
        eng.dma_start(out=x8[:, k, :], in_=x8T[k * P : (k + 1) * P, :])
    # PE warm-up: ~3.4us of dummy matmuls on memset data while the first x8
    # chunks stream in, so the HAM clock-gate reaches 8/8 (2.4 GHz) before
    # the first real matmul instead of ~15 matmuls into phase H.
    wrm = const.tile([P, 5 * P], BF)
    nc.vector.memset(wrm, 0.0)
    ps_wrm = psum.tile([P, TC], F32, name="ps_wrm", tag="ps")
    for i in range(16):
        nc.tensor.matmul(
            ps_wrm, lhsT=wrm[:, 0:P], rhs=wrm[:, P : P + TC],
            start=True, stop=True,
        )
    # bf16 x shard (needed from phase U on) behind x8 on the same two queues
    for k in range(ND):
        eng = nc.sync if k % 2 == 0 else nc.gpsimd
        eng.dma_start(out=xtile[:, k, :], in_=xT[k * P : (k + 1) * P, :])
    xs = [xtile[:, kd, :] for kd in range(ND)]

    # ---- remaining constants (all needed >=150us in), gpsimd queue
    tri = const.tile([P, P], BF)
    nc.gpsimd.dma_start(out=tri, in_=io["tri"][:, :])
    bg2c = const.tile([P, NR], F32)
    nc.gpsimd.dma_start(out=bg2c, in_=io["bg2_col"][:, :])
    bvc = const.tile([P, NR], F32)
    nc.gpsimd.dma_start(out=bvc, in_=io["bv_col"][:, :])
    cw = const.tile([P, ND, 3], F32)
    nc.gpsimd.dma_start(out=cw, in_=io["conv_w2"][:, :, :])
    cb = const.tile([P, ND], F32)
    nc.gpsimd.dma_start(out=cb, in_=io["conv_b2"][:, :])
    oddc = const.tile([P, 1], F32)
    nc.gpsimd.dma_start(out=oddc, in_=io["odd"][:, :])
    ones8 = const.tile([P, NT], F32)
    nc.vector.memset(ones8, 1.0)
    # pre-warm the ACT sigmoid table (1.3us ACT_TABLE_LOAD) during phase H
    # so the first real G sigmoid doesn't pay it on the PSUM-recycle path.
    warm = const.tile([P, 1], F32)
    nc.scalar.activation(out=warm[0:1, 0:1], in_=ones8[0:1, 0:1], func=AF.Sigmoid)
    if not zu:
        buB = const.tile([P, R], BF)
        nc.gpsimd.dma_start(out=buB, in_=io["buB"][:, :])
    if not zg2:
        bg2B = const.tile([P, R], BF)
        nc.gpsimd.dma_start(out=bg2B, in_=io["bg2B"][:, :])

    # ---- phase H (fp8 DoubleRow): h8 [dh-part, t] = relu(Wg1 @ x^T + bg1)*2^SX
    # pair-outer with all 8 PSUM banks; each DR matmul contracts two 128-d
    # chunks at bf16-matmul cost. psh lands at 2^(SX+SW); the 2^-SW rescale
    # rides the evacuation (DVE when bg1==0, ACT Relu bias/scale otherwise).
    h8 = hpool.tile([P, NDH, T], F8, name="h8", tag="h")
    psh = [[psum.tile([P, TC], F32, name=f"psh{kdh}{c}", tag="ps")
            for c in range(NTC)] for kdh in range(NDH)]
    for p in range(ND // 2):
        for kdh in range(NDH):
            for c in range(NTC):
                nc.tensor.matmul(
                    psh[kdh][c],
                    lhsT=wg18[:, 2 * p : 2 * p + 2, kdh * P : (kdh + 1) * P],
                    rhs=x8[:, 2 * p : 2 * p + 2, c * TC : (c + 1) * TC],
                    start=(p == 0),
                    stop=(p == ND // 2 - 1),
                    perf_mode=DR,
                )
    for kdh in range(NDH):
        for c in range(NTC):
            # evacuations split across DVE and ACT: phase G's first matmuls
            # reuse these PSUM banks, so halving the serial drain latency
            # starts G ~2us sooner.
            if zbg1 and c == 0:
                nc.vector.tensor_scalar(
                    h8[:, kdh, c * TC : (c + 1) * TC],
                    psh[kdh][c],
                    2.0 ** -SW,
                    0.0,
                    op0=ALU.mult,
                    op1=ALU.max,
                )
            else:
                nc.scalar.activation(
                    out=h8[:, kdh, c * TC : (c + 1) * TC], in_=psh[kdh][c],
                    func=AF.Relu,
                    bias=0.0 if zbg1 else bg1sc[:, kdh : kdh + 1],
                    scale=2.0 ** -SW,
                )

    # ---- phase G (fp8 DoubleRow): gn[t] [t-part, r] = sigmoid(h @ Wg2^T (+ bg2))
    gn = [gnp.tile([P, R], BF, name=f"gn{t}", tag="gn") for t in range(NT)]
    for rc in range(NRC):
        for h2 in range(2):
            psg = [psum.tile([P, TC], F32, name=f"psg{tt}", tag="ps")
                   for tt in range(NT // 2)]
            for p in range(NDH // 2):
                for tt in range(NT // 2):
                    t = h2 * (NT // 2) + tt
                    nc.tensor.matmul(
                        psg[tt],
                        lhsT=h8[:, 2 * p : 2 * p + 2, t * P : (t + 1) * P],
                        rhs=wg28[:, 2 * p : 2 * p + 2, rc * TC : (rc + 1) * TC],
                        start=(p == 0),
                        stop=(p == NDH // 2 - 1),
                        perf_mode=DR,
                    )
            for tt in range(NT // 2):
                t = h2 * (NT // 2) + tt
                if not zg2:
                    nc.vector.tensor_add(
                        out=psg[tt], in0=psg[tt],
                        in1=bg2B[:, rc * TC : (rc + 1) * TC],
                    )
                nc.scalar.activation(
                    out=gn[t][:, rc * TC : (rc + 1) * TC], in_=psg[tt],
                    func=AF.Sigmoid, scale=2.0 ** -(SX + SW),
                )

    # ---- phase U: ug[t] = (x @ Wu^T (+ bu)) * gn   [t-part, r]
    # ug tiles are [128, 1024]: two 512-wide rc chunks per tile.
    ugt = [[ugp.tile([P, 2 * TC], BF, name=f"ug_{t}_{rp}", tag="ug")
            for rp in range(2)] for t in range(NT)]
    bs_dram = dram.tile([P, NR], F32)
    for rc in range(NRC):
        psu = [psum.tile([P, TC], F32, name=f"psu{t}", tag="ps") for t in range(NT)]
        for k in range(ND):
            wut = wutp.tile([P, TC], BF, name="wut", tag="wut")
            nc.sync.dma_start(
                out=wut, in_=WuT[k * P : (k + 1) * P, rc * TC : (rc + 1) * TC]
            )
            for t in range(NT):
                nc.tensor.matmul(
                    psu[t],
                    lhsT=xs[k][:, 1 + t * P : 1 + t * P + P],
                    rhs=wut,
                    start=(k == 0),
                    stop=(k == ND - 1),
                )
        for t in range(NT):
            dst = ugt[t][rc // 2][:, (rc % 2) * TC : (rc % 2 + 1) * TC]
            if not zu:
                nc.vector.tensor_add(
                    out=psu[t], in0=psu[t],
                    in1=buB[:, rc * TC : (rc + 1) * TC],
                )
            nc.vector.tensor_mul(
                out=dst, in0=psu[t], in1=gn[t][:, rc * TC : (rc + 1) * TC],
            )

    # ---- phase CS (interleaved into GT/V below): block-local cumsum per
    # r-tile; extract block sums. intra[rk] lives in the gn pool (pairs).
    intra_t = [gnp.tile([P, R], BF, name=f"in{i}", tag="gn") for i in range(NT)]
    intra = [intra_t[rk // 2][:, (rk % 2) * T : (rk % 2 + 1) * T] for rk in range(NR)]
    bs_sb = const.tile([P, NR], F32)
    pbs = []
    deferred = []

    def flush_copy():
        # split the two fat PSUM->SBUF evacuations across ACT and DVE: the
        # PSUM pair recycles ~2x sooner, so the PE stalls less waiting for
        # free banks (ACT is ~9% busy here).
        frk, fps = deferred.pop(0)
        nc.scalar.activation(out=intra[frk][:, 0:TC], in_=fps[0], func=AF.Copy)
        nc.vector.tensor_copy(out=intra[frk][:, TC:T], in_=fps[1])

    def emit_cs(rk):
        rc4, ri4 = rk // 4, rk % 4
        ugsl = lambda j: ugt[j][rc4 // 2][:, (rc4 % 2) * TC + ri4 * P :
                                          (rc4 % 2) * TC + (ri4 + 1) * P]
        pscs = [psum.tile([P, TC], F32, name=f"pscs{c}", tag="ps") for c in range(2)]
        for j in range(NT):
            nc.tensor.matmul(
                pscs[j // 4][:, (j % 4) * P : (j % 4 + 1) * P],
                lhsT=ugsl(j),
                rhs=tri,
                start=True,
                stop=True,
            )
        # Pb gathers + block-sum reduce FIRST (they gate the collective);
        # the fat intra evacuation copies are deferred 1 r-tile.
        pb = pbp.tile([P, NT + 1], F32, name=f"pb{rk}", tag="pb")
        nc.vector.memset(pb[:, 0:1], 0.0)
        nc.vector.tensor_copy(out=pb[:, 1:5], in_=pscs[0][:, P - 1 :: P])
        nc.vector.tensor_copy(out=pb[:, 5:9], in_=pscs[1][:, P - 1 :: P])
        nc.vector.tensor_reduce(
            out=bs_sb[:, rk : rk + 1], in_=pb[:, 1:9],
            axis=mybir.AxisListType.X, op=ALU.add,
        )
        pbs.append(pb)
        deferred.append((rk, pscs))
        if len(deferred) > 1:
            flush_copy()

    pr_dram = dram.tile([P, NR], F32)
    prs = const.tile([P, NR], F32)
    carry = const.tile([P, NR], F32)

    def emit_allreduce():
        # [P, NR] end to end: contiguous per-partition DMAs (cheap launches),
        # and the elementwise AllReduce doesn't care about the layout. On the
        # gpsimd queue so its wait-for-bs_sb doesn't block the sync queue's
        # wvt loads (the V matmuls' feed).
        nc.gpsimd.dma_start(out=bs_dram[:, :], in_=bs_sb)
        nc.gpsimd.collective_compute(
            "AllReduce",
            ALU.add,
            replica_groups=[[2 * i, 2 * i + 1] for i in range(NCORES // 2)],
            ins=[bs_dram[:, :].opt()],
            outs=[pr_dram[:, :].opt()],
        )
        nc.gpsimd.dma_start(out=prs, in_=pr_dram[:, :])

    # ---- phase GT/V (PE+ACT+DVE) with got recombination on GPSIMD
    vgts = [None] * NR
    gos = [ugp.tile([P, T], BF, name=f"go{rk}", tag="ug") for rk in range(NR)]

    def emit_gtv(rk):
        # transposed gates, fp8 DR recompute from the SAME quantized h8/wg28
        # tiles as phase G: gtt == gn^T exactly (same products, same order).
        gtt = gtp.tile([P, T], BF, name="gtt", tag="gtt")
        psgt = [psum.tile([P, TC], F32, name=f"psgt{c}", tag="ps") for c in range(NTC)]
        for p in range(NDH // 2):
            for c in range(NTC):
                nc.tensor.matmul(
                    psgt[c],
                    lhsT=wg28[:, 2 * p : 2 * p + 2, rk * P : (rk + 1) * P],
                    rhs=h8[:, 2 * p : 2 * p + 2, c * TC : (c + 1) * TC],
                    start=(p == 0),
                    stop=(p == NDH // 2 - 1),
                    perf_mode=DR,
                )
        for c in range(NTC):
            nc.scalar.activation(
                out=gtt[:, c * TC : (c + 1) * TC],
                in_=psgt[c],
                func=AF.Sigmoid,
                bias=bg2c[:, rk : rk + 1],
                scale=2.0 ** -(SX + SW),
            )
        vgt = vgp.tile([P, T], BF, name="vgt", tag="vgt")
        psv = [psum.tile([P, TC], F32, name=f"psv{c}", tag="ps") for c in range(NTC)]
        for kg in range(ND // 4):
            wvt = wvtp.tile([P, 4 * P], BF, name="wvt", tag="wvt")
            nc.sync.dma_start(out=wvt, in_=WvPre[:, kg, rk, :])
            for i in range(4):
                k = kg * 4 + i
                for c in range(NTC):
                    nc.tensor.matmul(
                        psv[c],
                        lhsT=wvt[:, i * P : (i + 1) * P],
                        rhs=xs[k][:, 1 + c * TC : 1 + (c + 1) * TC],
                        start=(k == 0),
                        stop=(k == ND - 1),
                    )
        for c in range(NTC):
            nc.vector.scalar_tensor_tensor(
                out=vgt[:, c * TC : (c + 1) * TC],
                in0=psv[c],
                scalar=bvc[:, rk : rk + 1],
                in1=gtt[:, c * TC : (c + 1) * TC],
                op0=ALU.add,
                op1=ALU.mult,
            )
        return vgt

    GATE = int(os.environ.get("K_GATE", "6"))

    def emit_got(rk):
        if rk == 0:
            # The tile scheduler reorders by dependency, not emission order,
            # so a bare carry chain would be scheduled as soon as the DVE has
            # a gap -- stalling the whole DVE FIFO until the AllReduce lands.
            # This dummy 1-element copy makes the carry tile depend on
            # vgt(GATE), pinning the chain behind GATE r-tiles of GT/V work.
            nc.vector.tensor_copy(out=carry[0:1, 0:1], in_=vgts[GATE][0:1, 0:1])
            nc.vector.tensor_sub(out=carry, in0=prs, in1=bs_sb)
            nc.vector.tensor_scalar_mul(carry, carry, oddc[:, 0:1])
        # P_tot[:, j] = carry + sum_{j'<j} bsum_j'
        pt = ptp.tile([P, NT], F32, name="pt", tag="pt")
        nc.vector.tensor_tensor_scan(
            out=pt,
            data0=ones8,
            data1=pbs[rk][:, 0:NT],
            initial=carry[:, rk : rk + 1],
            op0=ALU.mult,
            op1=ALU.add,
        )
        for j in range(NT):
            nc.vector.scalar_tensor_tensor(
                out=gos[rk][:, j * P : (j + 1) * P],
                in0=intra[rk][:, j * P : (j + 1) * P],
                scalar=pt[:, j : j + 1],
                in1=vgts[rk][:, j * P : (j + 1) * P],
                op0=ALU.add,
                op1=ALU.mult,
            )
        vgts[rk] = None

    # CS r-tiles ride the first 4 GT/V iterations (4 per iteration): the
    # CS matmuls are tiny (N=128) and their PSUM drains are DVE/ACT-bound,
    # so standalone they leave the PE sparse AND let the HAM re-throttle
    # the clock; inside GT/V the PE stays dense and warm.
    for i in range(NR + DG):
        if i < NR:
            vgts[i] = emit_gtv(i)
            if i < 2:
                for rk in range(8 * i, 8 * i + 8):
                    emit_cs(rk)
                if i == 1:
                    while deferred:
                        flush_copy()
                    emit_allreduce()
        j = i - DG
        if j >= 0:
            emit_got(j)

    # ---- phase UV + conv epilogue: outT[d, t] = got-proj + conv + conv_b
    # The conv term ct depends only on x, so it is emitted BEFORE the kd's
    # matmuls: the DVE computes it while the PE accumulates, and the last
    # tile's epilogue is just one add + DMA after the final matmul.
    for kd in range(ND):
        wu2 = []
        for rg in range(NR // 4):
            wu24 = wu2p.tile([P, 4 * P], BF, name="wu24", tag="wu24")
            nc.sync.dma_start(out=wu24, in_=Wu2Pre[:, rg, kd, :])
            wu2.append(wu24)
        cts = []
        for c in range(NTC):
            ct = ctp.tile([P, TC], F32, name="ct", tag="ct")
            # dummy dep: ct chains depend only on x, and the scheduler would
            # otherwise run all 32 of them right after x lands -- ahead of
            # the phase-H relu evacuations, starving phase G. Gating each on
            # gos[kd] pins them into the UV phase where the DVE is idle.
            nc.vector.tensor_copy(out=ct[0:1, 0:1], in_=gos[kd][0:1, 0:1])
            nc.vector.tensor_scalar(
                ct,
                xs[kd][:, c * TC : c * TC + TC],
                cw[:, kd, 0:1],
                cb[:, kd : kd + 1],
                op0=ALU.mult,
                op1=ALU.add,
            )
            nc.vector.scalar_tensor_tensor(
                out=ct,
                in0=xs[kd][:, c * TC + 1 : c * TC + 1 + TC],
                scalar=cw[:, kd, 1:2],
                in1=ct,
                op0=ALU.mult,
                op1=ALU.add,
            )
            nc.vector.scalar_tensor_tensor(
                out=ct,
                in0=xs[kd][:, c * TC + 2 : c * TC + 2 + TC],
                scalar=cw[:, kd, 2:3],
                in1=ct,
                op0=ALU.mult,
                op1=ALU.add,
            )
            cts.append(ct)
        psuv = [psum.tile([P, TC], F32, name=f"psuv{c}", tag="ps") for c in range(NTC)]
        for rk in range(NR):
            for c in range(NTC):
                nc.tensor.matmul(
                    psuv[c],
                    lhsT=wu2[rk // 4][:, (rk % 4) * P : (rk % 4 + 1) * P],
                    rhs=gos[rk][:, c * TC : (c + 1) * TC],
                    start=(rk == 0),
                    stop=(rk == NR - 1),
                )
        for c in range(NTC):
            ob = outp.tile([P, TC], F32, name="ob", tag="ob")
            nc.vector.tensor_add(out=ob, in0=psuv[c], in1=cts[c])
            if kd >= ND - 2:
                # split the drain of the last tiles across four queues: the
                # final out DMA is the kernel's critical tail.
                for qi, eng in enumerate((nc.sync, nc.gpsimd, nc.scalar, nc.gpsimd)):
                    eng.dma_start(
                        out=outT[kd * P + qi * 32 : kd * P + (qi + 1) * 32,
                                 c * TC : (c + 1) * TC],
                        in_=ob[qi * 32 : (qi + 1) * 32, :],
                    )
            else:
                nc.sync.dma_start(
                    out=outT[kd * P : (kd + 1) * P, c * TC : (c + 1) * TC], in_=ob
                )
    ctx.close()


def _split_multi_waits(nc):
    """The walrus build in this env allows only ONE attached sync-wait per
    instruction; hoist extra waits onto standalone InstEventSemaphore ops
    inserted just before, on the same engine (semantically identical)."""
    import bass_rust

    n = 0
    for blk in nc.m.functions[0].blocks:
        changed = False
        out = []
        for ins in blk.instructions:
            si = getattr(ins, "sync_info", None)
            if si is not None and len(si.on_wait) > 1:
                waits = list(si.on_wait)
                for w in waits[:-1]:
                    ev = mybir.InstEventSemaphore(name=f"WSPLIT-{n}", ins=[], outs=[])
                    n += 1
                    ev.engine = ins.engine
                    ev.sync_info = bass_rust.SyncInfo(on_wait=[w], on_update=[])
                    out.append(ev)
                ins.sync_info = bass_rust.SyncInfo(
                    on_wait=[waits[-1]], on_update=list(si.on_update)
                )
                changed = True
            out.append(ins)
        if changed:
            try:
                blk.instructions[:] = out
            except TypeError:
                blk.instructions = out
    return n


def _build(zu, zg2, zbg1):
    nc = bass.Bass(num_devices=NCORES)
    io = {}
    io["xT"] = nc.declare_dram_parameter("xT", [D, TH], BF, False)
    io["x8T"] = nc.declare_dram_parameter("x8T", [D, T], F8, False)
    io["WuT"] = nc.declare_dram_parameter("WuT", [D, R], BF, False)
    io["WvPre"] = nc.declare_dram_parameter("WvPre", [P, 4, NR, 4 * P], BF, False)
    io["Wu2Pre"] = nc.declare_dram_parameter("Wu2Pre", [P, NR // 4, ND, 4 * P], BF, False)
    io["Wg18"] = nc.declare_dram_parameter("Wg18", [P, ND * DH], F8, False)
    io["Wg28"] = nc.declare_dram_parameter("Wg28", [P, NDH * R], F8, False)
    io["tri"] = nc.declare_dram_parameter("tri", [P, P], BF, False)
    io["bg1s_col"] = nc.declare_dram_parameter("bg1s_col", [P, NDH], F32, False)
    io["bg2_col"] = nc.declare_dram_parameter("bg2_col", [P, NR], F32, False)
    io["bv_col"] = nc.declare_dram_parameter("bv_col", [P, NR], F32, False)
    io["conv_w2"] = nc.declare_dram_parameter("conv_w2", [P, ND, 3], F32, False)
    io["conv_b2"] = nc.declare_dram_parameter("conv_b2", [P, ND], F32, False)
    io["odd"] = nc.declare_dram_parameter("odd", [P, 1], F32, False)
    if not zu:
        io["buB"] = nc.declare_dram_parameter("buB", [P, R], BF, False)
    if not zg2:
        io["bg2B"] = nc.declare_dram_parameter("bg2B", [P, R], BF, False)
    io["outT"] = nc.declare_dram_parameter("outT", [D, T], F32, True)
    with tile.TileContext(nc, num_cores=NCORES) as tc:
        io["tc"] = tc
        _emit(nc, io, zu, zg2, zbg1)
    _split_multi_waits(nc)
    return nc


_NC_CACHE = {}


def _get_nc(zu, zg2, zbg1):
    key = (zu, zg2, zbg1)
    if key not in _NC_CACHE:
        _NC_CACHE[key] = _build(zu, zg2, zbg1)
    return _NC_CACHE[key]


def _q8(t, s):
    """TRN e4m3 quantization with power-of-2 scale 2^s (clip to max normal)."""
    return np.clip(np.asarray(t, np.float32) * (2.0 ** s), -240.0, 240.0).astype(
        ml_dtypes.float8_e4m3
    )


def _prep_in_maps(x, Wu, bu, Wv, bv, Wg1, bg1, Wg2, bg2, conv_w, conv_b):
    bf = ml_dtypes.bfloat16
    f32 = np.float32
    x = np.asarray(x, f32)
    bu = np.asarray(bu, f32)
    bg1 = np.asarray(bg1, f32)
    bg2 = np.asarray(bg2, f32)
    zu = not bu.any()
    zg2 = not bg2.any()
    zbg1 = not bg1.any()
    WuT = np.asarray(Wu, f32).T
    WvT = np.asarray(Wv, f32).T
    # pre-shuffled DMA layouts: one contiguous run per SBUF partition.
    # WvPre[p, kg, rk, i*128+c] = WvT[kg*512 + i*128 + p, rk*128 + c]
    WvPre = np.ascontiguousarray(
        WvT.reshape(4, 4, P, NR, P).transpose(2, 0, 3, 1, 4).reshape(P, 4, NR, 4 * P)
    ).astype(bf)
    # Wu2Pre[p, rg, kd, i*128+c] = WuT[rg*512 + i*128 + p, kd*128 + c]
    Wu2Pre = np.ascontiguousarray(
        WuT.reshape(4, 4, P, ND, P).transpose(2, 0, 3, 1, 4).reshape(P, 4, ND, 4 * P)
    ).astype(bf)
    shared = dict(
        WuT=WuT.astype(bf),
        WvPre=WvPre,
        Wu2Pre=Wu2Pre,
        Wg18=np.ascontiguousarray(
            _q8(np.asarray(Wg1, f32).T, SW).reshape(ND, P, DH).transpose(1, 0, 2)
            .reshape(P, ND * DH)
        ),
        Wg28=np.ascontiguousarray(
            _q8(np.asarray(Wg2, f32).T, SW).reshape(NDH, P, R).transpose(1, 0, 2)
            .reshape(P, NDH * R)
        ),
        tri=(np.arange(P)[:, None] <= np.arange(P)[None, :]).astype(bf),
        bg1s_col=np.ascontiguousarray((bg1 * (2.0 ** SX)).reshape(NDH, P).T),
        bg2_col=np.ascontiguousarray(bg2.reshape(NR, P).T),
        bv_col=np.ascontiguousarray(np.asarray(bv, f32).reshape(NR, P).T),
        conv_w2=np.ascontiguousarray(
            np.asarray(conv_w, f32)[:, 0, :].reshape(ND, P, 3).transpose(1, 0, 2)
        ),
        conv_b2=np.ascontiguousarray(np.asarray(conv_b, f32).reshape(ND, P).T),
    )
    if not zu:
        shared["buB"] = np.broadcast_to(bu.astype(bf), (P, R)).copy()
    if not zg2:
        # pre-added into the psg PSUM (which sits at scale 2^(SX+SW))
        shared["bg2B"] = np.broadcast_to(
            (bg2 * (2.0 ** (SX + SW))).astype(bf), (P, R)
        ).copy()
    xflat = x.reshape(B * L, D)
    in_maps = []
    for c in range(NCORES):
        xh = np.zeros((TH, D), f32)
        xh[1 : T + 1] = xflat[c * T : (c + 1) * T]
        if c % 2 == 1:
            xh[0] = xflat[c * T - 1]
        else:
            xh[T + 1] = xflat[(c + 1) * T]
        odd = np.full((P, 1), float(c % 2), f32)
        in_maps.append(
            dict(
                shared,
                xT=xh.T.astype(bf),
                x8T=_q8(xflat[c * T : (c + 1) * T].T, SX),
                odd=odd,
            )
        )
    return in_maps, zu, zg2, zbg1


def _assemble(results):
    out = np.empty((B * L, D), np.float32)
    for c in range(NCORES):
        out[c * T : (c + 1) * T] = np.asarray(results[c]["outT"]).T
    return out.reshape(B, L, D)


def kernel(x, Wu, bu, Wv, bv, Wg1, bg1, Wg2, bg2, conv_w, conv_b):
    in_maps, zu, zg2, zbg1 = _prep_in_maps(
        x, Wu, bu, Wv, bv, Wg1, bg1, Wg2, bg2, conv_w, conv_b
    )
    res = run_bass_kernel_spmd(
        _get_nc(zu, zg2, zbg1), in_maps, core_ids=list(range(NCORES))
    )
    return _assemble(res.results)


def run_traced(inputs):
    """Profiled run: returns (output, exec_time_ns)."""
    in_maps, zu, zg2, zbg1 = _prep_in_maps(**inputs)
    res = run_bass_kernel_spmd(
        _get_nc(zu, zg2, zbg1), in_maps, core_ids=list(range(NCORES)), trace=True
    )
    return _assemble(res.results), res.exec_time_ns



# revision 30
# speedup vs baseline: 1.0066x; 1.0066x over previous
"""Trainium2 Bass kernel for AdaptiveSemiseparableLayer (v3).

v3: the whole gate MLP (H, G, and the transposed-gate recompute GT) runs
in fp8 e4m3 with MatmulPerfMode.DoubleRow — each matmul contracts 256
inputs in the time a bf16 matmul contracts 128, halving the PE cycles of
those phases (384 -> 192 matmul-equivalents). x/Wg1/Wg2 are quantized
host-side with static power-of-2 scales (x*2^5, W*2^10); the rescale
rides the existing evacuation ops (DVE tensor_scalar for relu, ACT
activation(scale=...) for the sigmoids). GT reuses the same quantized
h8/wg28 tiles as G, so gtt == gn^T exactly and the fp8 error is NOT paid
twice. Measured end-to-end rel err ~1.2e-2 (gate 2e-2). U/V/UV stay bf16:
fp8 there measures 3.7e-2..6.5e-2 — over the gate.

Reference computation (B=4, L=2048, D=R=2048, DH=512):
    t_out = depthwise_conv1d(x, conv_w, k=3) + conv_b
    u = x @ Wu.T + bu ; v = x @ Wv.T + bv
    gates = sigmoid(relu(x @ Wg1.T + bg1) @ Wg2.T + bg2)
    cs = cumsum(u * gates, axis=seq)
    out = t_out + (cs * (v * gates)) @ Wu.T

Sharding: sequence-parallel; 8192 tokens -> 8 contiguous 1024-token blocks
(one per core; each block lies inside one batch row). The only cross-core
dependency is the cumsum carry: odd core c needs core c-1's column sums.
That is a pairwise AllReduce(add) over (R,) = 8KB; each core then forms
carry = (pair_sum - own_sum) * is_odd on GPSIMD.

Changes vs the original kernel (637 us -> ~465 us):
  - two-level cumsum: 8 independent 128-wide triangular matmuls per r-tile
    (8 MMs, 1024 PE cycles) instead of the 12-MM trapezoid; the 8-block
    prefix is recombined per 128-token window on DVE via a scan +
    scalar_tensor_tensor (cs + P) * vg.
  - block sums come from the intra-cumsum PSUM's last columns (DVE strided
    gather + reduce) -- the 128 free-dim-1 block-sum matmuls are gone.
  - the carry selection matmuls (mask MMs) are gone (pairwise AllReduce
    over core pairs + (pair_sum - own_sum) * is_odd).
  - all bias matmuls are gone: bg1 rides a DVE tensor_scalar (add,max),
    bg2(GT) rides the ACT activation bias, bv rides a
    scalar_tensor_tensor, bu/bg2(G-natural) have a DVE broadcast-add
    fallback variant only when nonzero (they are zero here).
  - the tile scheduler reorders by dependency, not emission order, so all
    carry-dependent DVE work (and the x-only conv ct chains) is pinned via
    dummy 1-element WAW copies: the carry chain behind vgt(GATE), each
    kd's ct chain behind gos[kd]. Without these the DVE FIFO stalls on the
    ~35us AllReduce latency (or runs 32 ct chains before phase H's relus).
  - H is k-outer across all 8 PSUM banks so the 16 x chunks are consumed
    at DMA arrival pace; Wg1/Wg2 preloaded whole on the ACT DMA queue.
  - gn/intra and ug/gos share SBUF pools (phase-disjoint reuse); the
    final output tiles drain over three DMA queues.

Layouts (per core, T=1024 local tokens):
    xT    [D, T+2]  bf16   transposed shard with conv halo columns
    h     [dh-part, t]     (4 tiles)
    gn    [t-part, r]      (8 tiles)   natural gates (for ug)
    ug    [t-part, r]      bf16, lhsT of the intra cumsum matmuls
    intra [r-part, t]      bf16, block-local inclusive cumsum
    gtt/vgt [r-part, t]    transposed gates / gated v
    got   [r-part, t]      (cs_local + prefix + carry) * vg
    outT  [d-part, t]      final, conv fused in epilogue
"""

import numpy as np
import ml_dtypes
from contextlib import ExitStack

import concourse.bass as bass
import concourse.mybir as mybir
import concourse.tile as tile
from concourse.bass_utils import run_bass_kernel_spmd

P = 128
B, L, D = 4, 2048, 2048
R, DH = 2048, 512
NCORES = 8
T = (B * L) // NCORES          # 1024 tokens per core
TH = T + 2                     # with conv halo
ND, NR, NDH, NT = D // P, R // P, DH // P, T // P
TC = 512                       # matmul free-dim chunk (one PSUM bank of f32)
NTC = T // TC                  # 2
NRC = R // TC                  # 4
BF = mybir.dt.bfloat16
F8 = mybir.dt.float8e4
F32 = mybir.dt.float32
AF = mybir.ActivationFunctionType
ALU = mybir.AluOpType
DR = mybir.MatmulPerfMode.DoubleRow

# fp8 static power-of-2 scales for the gate MLP (x, Wg1/Wg2 quantized to
# TRN e4m3, max-normal 240): psh/psg land at 2^(SX+SW); the rescale rides
# the evacuation op (DVE mult / ACT activation scale).
SX = 5     # x, h are scaled by 2^SX
SW = 10    # Wg1, Wg2 scaled by 2^SW

import os
DG = int(os.environ.get("K_DG", "12"))    # got lag behind gtv emission


def _emit(nc, io, zu, zg2, zbg1):
    """zu/zg2/zbg1: True when bu / bg2 / bg1 are all-zero (skip bias work)."""
    ctx = ExitStack()
    tc = io["tc"]
    pool = lambda name, bufs, **kw: ctx.enter_context(
        tc.tile_pool(name=name, bufs=bufs, **kw)
    )
    const = pool("const", 1)
    xpool = pool("xpool", 1)
    hpool = pool("hpool", 1)       # single fp8 h tile [P, NDH, T]
    gnp = pool("gnp", NT)          # gn tiles; reused for intra pairs
    ugp = pool("ugp", 2 * NT)      # ug tiles [128,1024]; reused for gos
    pbp = pool("pbp", NR)          # Pb [128, 9] f32 per rk
    ptp = pool("ptp", 3)           # Ptot transient
    gtp = pool("gtp", 2)
    vgp = pool("vgp", DG + 2)
    wutp = pool("wutp", 6)
    wvtp = pool("wvtp", 8)
    wu2p = pool("wu2p", 5)
    outp = pool("outp", 2)
    ctp = pool("ctp", 4)
    psum = pool("psum", 8, space="PSUM")
    dram = pool("dram", 1, space="DRAM")

    xT, WuT = (io[k] for k in ["xT", "WuT"])
    WvPre, Wu2Pre = io["WvPre"], io["Wu2Pre"]
    x8T, Wg18, Wg28 = (io[k] for k in ["x8T", "Wg18", "Wg28"])
    outT = io["outT"]

    # ---- critical-path loads first: fp8 Wg1 + fp8 x shard (phase H inputs).
    # All weight/const layouts are pre-shuffled HOST-side so every DMA here
    # is one contiguous run per partition (~128 descriptors): the sequencer
    # DIRECT2D launch cost stays ~0.6us instead of 2-4us, which matters
    # because launches serialize on the issuing engine's queue (the ACT
    # queue must be clear before the phase-G sigmoids can run).
    wg18 = const.tile([P, ND, DH], F8)
    x8 = xpool.tile([P, ND, T], F8, name="x8", tag="x8")
    xtile = xpool.tile([P, ND, TH], BF, name="xt", tag="xt")
    # x8 even chunks on sync, odd on gpsimd: each H pair (2 chunks) arrives
    # at two-queue rate, ~2x the pair consumption rate of the DR matmuls.
    nc.scalar.dma_start(out=wg18[:, 0:2, :], in_=Wg18[:, 0 : 2 * DH])
    for k in (0, 1):
        eng = nc.sync if k % 2 == 0 else nc.gpsimd
        eng.dma_start(out=x8[:, k, :], in_=x8T[k * P : (k + 1) * P, :])
    if not zbg1:
        bg1sc = const.tile([P, NDH], F32)
        nc.scalar.dma_start(out=bg1sc, in_=io["bg1s_col"][:, :])
    nc.scalar.dma_start(out=wg18[:, 2:8, :], in_=Wg18[:, 2 * DH : 8 * DH])
    nc.scalar.dma_start(out=wg18[:, 8:16, :], in_=Wg18[:, 8 * DH : 16 * DH])
    for k in range(2, 10):
        eng = nc.sync if k % 2 == 0 else nc.gpsimd
        eng.dma_start(out=x8[:, k, :], in_=x8T[k * P : (k + 1) * P, :])
    # fp8 Wg2 next on scalar (needed at phase G, ~20us in); after this the
    # ACT queue is free for the sigmoids.
    wg28 = const.tile([P, NDH, R], F8)
    nc.scalar.dma_start(out=wg28, in_=Wg28[:, :])
    for k in range(10, ND):
        eng = nc.sync if k % 2 == 0 else nc.gpsimd
        eng.dma_start(out=x8[:, k, :], in_=x8T[k * P : (k + 1) * P, :])
    # PE warm-up: ~3.4us of dummy matmuls on memset data while the first x8
    # chunks stream in, so the HAM clock-gate reaches 8/8 (2.4 GHz) before
    # the first real matmul instead of ~15 matmuls into phase H.
    wrm = const.tile([P, 5 * P], BF)
    nc.vector.memset(wrm, 0.0)
    ps_wrm = psum.tile([P, TC], F32, name="ps_wrm", tag="ps")
    for i in range(16):
        nc.tensor.matmul(
            ps_wrm, lhsT=wrm[:, 0:P], rhs=wrm[:, P : P + TC],
            start=True, stop=True,
        )
    # bf16 x shard (needed from phase U on) behind x8 on the same two queues
    for k in range(ND):
        eng = nc.sync if k % 2 == 0 else nc.gpsimd
        eng.dma_start(out=xtile[:, k, :], in_=xT[k * P : (k + 1) * P, :])
    xs = [xtile[:, kd, :] for kd in range(ND)]

    # ---- remaining constants (all needed >=150us in), gpsimd queue
    tri = const.tile([P, P], BF)
    nc.gpsimd.dma_start(out=tri, in_=io["tri"][:, :])
    bg2c = const.tile([P, NR], F32)
    nc.gpsimd.dma_start(out=bg2c, in_=io["bg2_col"][:, :])
    bvc = const.tile([P, NR], F32)
    nc.gpsimd.dma_start(out=bvc, in_=io["bv_col"][:, :])
    cw = const.tile([P, ND, 3], F32)
    nc.gpsimd.dma_start(out=cw, in_=io["conv_w2"][:, :, :])
    cb = const.tile([P, ND], F32)
    nc.gpsimd.dma_start(out=cb, in_=io["conv_b2"][:, :])
    oddc = const.tile([P, 1], F32)
    nc.gpsimd.dma_start(out=oddc, in_=io["odd"][:, :])
    ones8 = const.tile([P, NT], F32)
    nc.vector.memset(ones8, 1.0)
    # pre-warm the ACT sigmoid table (1.3us ACT_TABLE_LOAD) during phase H
    # so the first real G sigmoid doesn't pay it on the PSUM-recycle path.
    warm = const.tile([P, 1], F32)
    nc.scalar.activation(out=warm[0:1, 0:1], in_=ones8[0:1, 0:1], func=AF.Sigmoid)
    if not zu:
        buB = const.tile([P, R], BF)
        nc.gpsimd.dma_start(out=buB, in_=io["buB"][:, :])
    if not zg2:
        bg2B = const.tile([P, R], BF)
        nc.gpsimd.dma_start(out=bg2B, in_=io["bg2B"][:, :])

    # ---- phase H (fp8 DoubleRow): h8 [dh-part, t] = relu(Wg1 @ x^T + bg1)*2^SX
    # pair-outer with all 8 PSUM banks; each DR matmul contracts two 128-d
    # chunks at bf16-matmul cost. psh lands at 2^(SX+SW); the 2^-SW rescale
    # rides the evacuation (DVE when bg1==0, ACT Relu bias/scale otherwise).
    h8 = hpool.tile([P, NDH, T], F8, name="h8", tag="h")
    psh = [[psum.tile([P, TC], F32, name=f"psh{kdh}{c}", tag="ps")
            for c in range(NTC)] for kdh in range(NDH)]
    for p in range(ND // 2):
        for kdh in range(NDH):
            for c in range(NTC):
                nc.tensor.matmul(
                    psh[kdh][c],
                    lhsT=wg18[:, 2 * p : 2 * p + 2, kdh * P : (kdh + 1) * P],
                    rhs=x8[:, 2 * p : 2 * p + 2, c * TC : (c + 1) * TC],
                    start=(p == 0),
                    stop=(p == ND // 2 - 1),
                    perf_mode=DR,
                )
    for kdh in range(NDH):
        for c in range(NTC):
            # evacuations split across DVE and ACT: phase G's first matmuls
            # reuse these PSUM banks, so halving the serial drain latency
            # starts G ~2us sooner.
            if zbg1 and c == 0:
                nc.vector.tensor_scalar(
                    h8[:, kdh, c * TC : (c + 1) * TC],
                    psh[kdh][c],
                    2.0 ** -SW,
                    0.0,
                    op0=ALU.mult,
                    op1=ALU.max,
                )
            else:
                nc.scalar.activation(
                    out=h8[:, kdh, c * TC : (c + 1) * TC], in_=psh[kdh][c],
                    func=AF.Relu,
                    bias=0.0 if zbg1 else bg1sc[:, kdh : kdh + 1],
                    scale=2.0 ** -SW,
                )

    # ---- phase G (fp8 DoubleRow): gn[t] [t-part, r] = sigmoid(h @ Wg2^T (+ bg2))
    gn = [gnp.tile([P, R], BF, name=f"gn{t}", tag="gn") for t in range(NT)]
    for rc in range(NRC):
        for h2 in range(2):
            psg = [psum.tile([P, TC], F32, name=f"psg{tt}", tag="ps")
                   for tt in range(NT // 2)]
            for p in range(NDH // 2):
                for tt in range(NT // 2):
                    t = h2 * (NT // 2) + tt
                    nc.tensor.matmul(
                        psg[tt],
                        lhsT=h8[:, 2 * p : 2 * p + 2, t * P : (t + 1) * P],
                        rhs=wg28[:, 2 * p : 2 * p + 2, rc * TC : (rc + 1) * TC],
                        start=(p == 0),
                        stop=(p == NDH // 2 - 1),
                        perf_mode=DR,
                    )
            for tt in range(NT // 2):
                t = h2 * (NT // 2) + tt
                if not zg2:
                    nc.vector.tensor_add(
                        out=psg[tt], in0=psg[tt],
                        in1=bg2B[:, rc * TC : (rc + 1) * TC],
                    )
                nc.scalar.activation(
                    out=gn[t][:, rc * TC : (rc + 1) * TC], in_=psg[tt],
                    func=AF.Sigmoid, scale=2.0 ** -(SX + SW),
                )

    # ---- phase U: ug[t] = (x @ Wu^T (+ bu)) * gn   [t-part, r]
    # ug tiles are [128, 1024]: two 512-wide rc chunks per tile.
    ugt = [[ugp.tile([P, 2 * TC], BF, name=f"ug_{t}_{rp}", tag="ug")
            for rp in range(2)] for t in range(NT)]
    bs_dram = dram.tile([P, NR], F32)
    for rc in range(NRC):
        psu = [psum.tile([P, TC], F32, name=f"psu{t}", tag="ps") for t in range(NT)]
        for k in range(ND):
            wut = wutp.tile([P, TC], BF, name="wut", tag="wut")
            nc.sync.dma_start(
                out=wut, in_=WuT[k * P : (k + 1) * P, rc * TC : (rc + 1) * TC]
            )
            for t in range(NT):
                nc.tensor.matmul(
                    psu[t],
                    lhsT=xs[k][:, 1 + t * P : 1 + t * P + P],
                    rhs=wut,
                    start=(k == 0),
                    stop=(k == ND - 1),
                )
        for t in range(NT):
            dst = ugt[t][rc // 2][:, (rc % 2) * TC : (rc % 2 + 1) * TC]
            if not zu:
                nc.vector.tensor_add(
                    out=psu[t], in0=psu[t],
                    in1=buB[:, rc * TC : (rc + 1) * TC],
                )
            nc.vector.tensor_mul(
                out=dst, in0=psu[t], in1=gn[t][:, rc * TC : (rc + 1) * TC],
            )

    # ---- phase CS (interleaved into GT/V below): block-local cumsum per
    # r-tile; extract block sums. intra[rk] lives in the gn pool (pairs).
    intra_t = [gnp.tile([P, R], BF, name=f"in{i}", tag="gn") for i in range(NT)]
    intra = [intra_t[rk // 2][:, (rk % 2) * T : (rk % 2 + 1) * T] for rk in range(NR)]
    bs_sb = const.tile([P, NR], F32)
    pbs = []
    deferred = []

    def flush_copy():
        # split the two fat PSUM->SBUF evacuations across ACT and DVE: the
        # PSUM pair recycles ~2x sooner, so the PE stalls less waiting for
        # free banks (ACT is ~9% busy here).
        frk, fps = deferred.pop(0)
        nc.scalar.activation(out=intra[frk][:, 0:TC], in_=fps[0], func=AF.Copy)
        nc.vector.tensor_copy(out=intra[frk][:, TC:T], in_=fps[1])

    def emit_cs(rk):
        rc4, ri4 = rk // 4, rk % 4
        ugsl = lambda j: ugt[j][rc4 // 2][:, (rc4 % 2) * TC + ri4 * P :
                                          (rc4 % 2) * TC + (ri4 + 1) * P]
        pscs = [psum.tile([P, TC], F32, name=f"pscs{c}", tag="ps") for c in range(2)]
        for j in range(NT):
            nc.tensor.matmul(
                pscs[j // 4][:, (j % 4) * P : (j % 4 + 1) * P],
                lhsT=ugsl(j),
                rhs=tri,
                start=True,
                stop=True,
            )
        # Pb gathers + block-sum reduce FIRST (they gate the collective);
        # the fat intra evacuation copies are deferred 1 r-tile.
        pb = pbp.tile([P, NT + 1], F32, name=f"pb{rk}", tag="pb")
        nc.vector.memset(pb[:, 0:1], 0.0)
        nc.vector.tensor_copy(out=pb[:, 1:5], in_=pscs[0][:, P - 1 :: P])
        nc.vector.tensor_copy(out=pb[:, 5:9], in_=pscs[1][:, P - 1 :: P])
        nc.vector.tensor_reduce(
            out=bs_sb[:, rk : rk + 1], in_=pb[:, 1:9],
            axis=mybir.AxisListType.X, op=ALU.add,
        )
        pbs.append(pb)
        deferred.append((rk, pscs))
        if len(deferred) > 1:
            flush_copy()

    pr_dram = dram.tile([P, NR], F32)
    prs = const.tile([P, NR], F32)
    carry = const.tile([P, NR], F32)

    def emit_allreduce():
        # [P, NR] end to end: contiguous per-partition DMAs (cheap launches),
        # and the elementwise AllReduce doesn't care about the layout. On the
        # gpsimd queue so its wait-for-bs_sb doesn't block the sync queue's
        # wvt loads (the V matmuls' feed).
        nc.gpsimd.dma_start(out=bs_dram[:, :], in_=bs_sb)
        nc.gpsimd.collective_compute(
            "AllReduce",
            ALU.add,
            replica_groups=[[2 * i, 2 * i + 1] for i in range(NCORES // 2)],
            ins=[bs_dram[:, :].opt()],
            outs=[pr_dram[:, :].opt()],
        )
        nc.gpsimd.dma_start(out=prs, in_=pr_dram[:, :])

    # ---- phase GT/V (PE+ACT+DVE) with got recombination on GPSIMD
    vgts = [None] * NR
    gos = [ugp.tile([P, T], BF, name=f"go{rk}", tag="ug") for rk in range(NR)]

    def emit_gtv(rk):
        # transposed gates, fp8 DR recompute from the SAME quantized h8/wg28
        # tiles as phase G: gtt == gn^T exactly (same products, same order).
        gtt = gtp.tile([P, T], BF, name="gtt", tag="gtt")
        psgt = [psum.tile([P, TC], F32, name=f"psgt{c}", tag="ps") for c in range(NTC)]
        for p in range(NDH // 2):
            for c in range(NTC):
                nc.tensor.matmul(
                    psgt[c],
                    lhsT=wg28[:, 2 * p : 2 * p + 2, rk * P : (rk + 1) * P],
                    rhs=h8[:, 2 * p : 2 * p + 2, c * TC : (c + 1) * TC],
                    start=(p == 0),
                    stop=(p == NDH // 2 - 1),
                    perf_mode=DR,
                )
        for c in range(NTC):
            nc.scalar.activation(
                out=gtt[:, c * TC : (c + 1) * TC],
                in_=psgt[c],
                func=AF.Sigmoid,
                bias=bg2c[:, rk : rk + 1],
                scale=2.0 ** -(SX + SW),
            )
        vgt = vgp.tile([P, T], BF, name="vgt", tag="vgt")
        psv = [psum.tile([P, TC], F32, name=f"psv{c}", tag="ps") for c in range(NTC)]
        for kg in range(ND // 4):
            wvt = wvtp.tile([P, 4 * P], BF, name="wvt", tag="wvt")
            nc.sync.dma_start(out=wvt, in_=WvPre[:, kg, rk, :])
            for i in range(4):
                k = kg * 4 + i
                for c in range(NTC):
                    nc.tensor.matmul(
                        psv[c],
                        lhsT=wvt[:, i * P : (i + 1) * P],
                        rhs=xs[k][:, 1 + c * TC : 1 + (c + 1) * TC],
                        start=(k == 0),
                        stop=(k == ND - 1),
                    )
        for c in range(NTC):
            nc.vector.scalar_tensor_tensor(
                out=vgt[:, c * TC : (c + 1) * TC],
                in0=psv[c],
                scalar=bvc[:, rk : rk + 1],
                in1=gtt[:, c * TC : (c + 1) * TC],
                op0=ALU.add,
                op1=ALU.mult,
            )
        return vgt

    GATE = int(os.environ.get("K_GATE", "6"))

    def emit_got(rk):
        if rk == 0:
            # The tile scheduler reorders by dependency, not emission order,
            # so a bare carry chain would be scheduled as soon as the DVE has
            # a gap -- stalling the whole DVE FIFO until the AllReduce lands.
            # This dummy 1-element copy makes the carry tile depend on
            # vgt(GATE), pinning the chain behind GATE r-tiles of GT/V work.
            nc.vector.tensor_copy(out=carry[0:1, 0:1], in_=vgts[GATE][0:1, 0:1])
            nc.vector.tensor_sub(out=carry, in0=prs, in1=bs_sb)
            nc.vector.tensor_scalar_mul(carry, carry, oddc[:, 0:1])
        # P_tot[:, j] = carry + sum_{j'<j} bsum_j'
        pt = ptp.tile([P, NT], F32, name="pt", tag="pt")
        nc.vector.tensor_tensor_scan(
            out=pt,
            data0=ones8,
            data1=pbs[rk][:, 0:NT],
            initial=carry[:, rk : rk + 1],
            op0=ALU.mult,
            op1=ALU.add,
        )
        for j in range(NT):
            nc.vector.scalar_tensor_tensor(
                out=gos[rk][:, j * P : (j + 1) * P],
                in0=intra[rk][:, j * P : (j + 1) * P],
                scalar=pt[:, j : j + 1],
                in1=vgts[rk][:, j * P : (j + 1) * P],
                op0=ALU.add,
                op1=ALU.mult,
            )
        vgts[rk] = None

    # CS r-tiles ride the first 4 GT/V iterations (4 per iteration): the
    # CS matmuls are tiny (N=128) and their PSUM drains are DVE/ACT-bound,
    # so standalone they leave the PE sparse AND let the HAM re-throttle
    # the clock; inside GT/V the PE stays dense and warm.
    for i in range(NR + DG):
        if i < NR:
            vgts[i] = emit_gtv(i)
            if i < 2:
                for rk in range(8 * i, 8 * i + 8):
                    emit_cs(rk)
                if i == 1:
                    while deferred:
                        flush_copy()
                    emit_allreduce()
        j = i - DG
        if j >= 0:
            emit_got(j)

    # ---- phase UV + conv epilogue: outT[d, t] = got-proj + conv + conv_b
    # The conv term ct depends only on x, so it is emitted BEFORE the kd's
    # matmuls: the DVE computes it while the PE accumulates, and the last
    # tile's epilogue is just one add + DMA after the final matmul.
    for kd in range(ND):
        wu2 = []
        for rg in range(NR // 4):
            wu24 = wu2p.tile([P, 4 * P], BF, name="wu24", tag="wu24")
            nc.sync.dma_start(out=wu24, in_=Wu2Pre[:, rg, kd, :])
            wu2.append(wu24)
        cts = []
        for c in range(NTC):
            ct = ctp.tile([P, TC], F32, name="ct", tag="ct")
            # dummy dep: ct chains depend only on x, and the scheduler would
            # otherwise run all 32 of them right after x lands -- ahead of
            # the phase-H relu evacuations, starving phase G. Gating each on
            # gos[kd] pins them into the UV phase where the DVE is idle.
            nc.vector.tensor_copy(out=ct[0:1, 0:1], in_=gos[kd][0:1, 0:1])
            nc.vector.tensor_scalar(
                ct,
                xs[kd][:, c * TC : c * TC + TC],
                cw[:, kd, 0:1],
                cb[:, kd : kd + 1],
                op0=ALU.mult,
                op1=ALU.add,
            )
            nc.vector.scalar_tensor_tensor(
                out=ct,
                in0=xs[kd][:, c * TC + 1 : c * TC + 1 + TC],
                scalar=cw[:, kd, 1:2],
                in1=ct,
                op0=ALU.mult,
                op1=ALU.add,
            )
            nc.vector.scalar_tensor_tensor(
                out=ct,
                in0=xs[kd][:, c * TC + 2 : c * TC + 2 + TC],
                scalar=cw[:, kd, 2:3],
                in1=ct,
                op0=ALU.mult,
                op1=ALU.add,
            )
            cts.append(ct)
        psuv = [psum.tile([P, TC], F32, name=f"psuv{c}", tag="ps") for c in range(NTC)]
        for rk in range(NR):
            for c in range(NTC):
                nc.tensor.matmul(
                    psuv[c],
                    lhsT=wu2[rk // 4][:, (rk % 4) * P : (rk % 4 + 1) * P],
                    rhs=gos[rk][:, c * TC : (c + 1) * TC],
                    start=(rk == 0),
                    stop=(rk == NR - 1),
                )
        for c in range(NTC):
            ob = outp.tile([P, TC], F32, name="ob", tag="ob")
            nc.vector.tensor_add(out=ob, in0=psuv[c], in1=cts[c])
            if kd >= ND - 2:
                # split the drain of the last tiles across four queues: the
                # final out DMA is the kernel's critical tail.
                for qi, eng in enumerate((nc.sync, nc.gpsimd, nc.scalar, nc.gpsimd)):
                    eng.dma_start(
                        out=outT[kd * P + qi * 32 : kd * P + (qi + 1) * 32,
                                 c * TC : (c + 1) * TC],
                        in_=ob[qi * 32 : (qi + 1) * 32, :],
                    )
            else:
                nc.sync.dma_start(
                    out=outT[kd * P : (kd + 1) * P, c * TC : (c + 1) * TC], in_=ob
                )
    ctx.close()


def _split_multi_waits(nc):
    """The walrus build in this env allows only ONE attached sync-wait per
    instruction; hoist extra waits onto standalone InstEventSemaphore ops
    inserted just before, on the same engine (semantically identical)."""
    import bass_rust

    n = 0
    for blk in nc.m.functions[0].blocks:
        changed = False
        out = []
        for ins in blk.instructions:
            si = getattr(ins, "sync_info", None)
            if si is not None and len(si.on_wait) > 1:
                waits = list(si.on_wait)
                for w in waits[:-1]:
                    ev = mybir.InstEventSemaphore(name=f"WSPLIT-{n}", ins=[], outs=[])
                    n += 1
                    ev.engine = ins.engine
                    ev.sync_info = bass_rust.SyncInfo(on_wait=[w], on_update=[])
                    out.append(ev)
                ins.sync_info = bass_rust.SyncInfo(
                    on_wait=[waits[-1]], on_update=list(si.on_update)
                )
                changed = True
            out.append(ins)
        if changed:
            try:
                blk.instructions[:] = out
            except TypeError:
                blk.instructions = out
    return n


def _build(zu, zg2, zbg1):
    nc = bass.Bass(num_devices=NCORES)
    io = {}
    io["xT"] = nc.declare_dram_parameter("xT", [D, TH], BF, False)
    io["x8T"] = nc.declare_dram_parameter("x8T", [D, T], F8, False)
    io["WuT"] = nc.declare_dram_parameter("WuT", [D, R], BF, False)
    io["WvPre"] = nc.declare_dram_parameter("WvPre", [P, 4, NR, 4 * P], BF, False)
    io["Wu2Pre"] = nc.declare_dram_parameter("Wu2Pre", [P, NR // 4, ND, 4 * P], BF, False)
    io["Wg18"] = nc.declare_dram_parameter("Wg18", [P, ND * DH], F8, False)
    io["Wg28"] = nc.declare_dram_parameter("Wg28", [P, NDH * R], F8, False)
    io["tri"] = nc.declare_dram_parameter("tri", [P, P], BF, False)
    io["bg1s_col"] = nc.declare_dram_parameter("bg1s_col", [P, NDH], F32, False)
    io["bg2_col"] = nc.declare_dram_parameter("bg2_col", [P, NR], F32, False)
    io["bv_col"] = nc.declare_dram_parameter("bv_col", [P, NR], F32, False)
    io["conv_w2"] = nc.declare_dram_parameter("conv_w2", [P, ND, 3], F32, False)
    io["conv_b2"] = nc.declare_dram_parameter("conv_b2", [P, ND], F32, False)
    io["odd"] = nc.declare_dram_parameter("odd", [P, 1], F32, False)
    if not zu:
        io["buB"] = nc.declare_dram_parameter("buB", [P, R], BF, False)
    if not zg2:
        io["bg2B"] = nc.declare_dram_parameter("bg2B", [P, R], BF, False)
    io["outT"] = nc.declare_dram_parameter("outT", [D, T], F32, True)
    with tile.TileContext(nc, num_cores=NCORES) as tc:
        io["tc"] = tc
        _emit(nc, io, zu, zg2, zbg1)
    _split_multi_waits(nc)
    return nc


_NC_CACHE = {}


def _get_nc(zu, zg2, zbg1):
    key = (zu, zg2, zbg1)
    if key not in _NC_CACHE:
        _NC_CACHE[key] = _build(zu, zg2, zbg1)
    return _NC_CACHE[key]


def _q8(t, s):
    """TRN e4m3 quantization with power-of-2 scale 2^s (clip to max normal)."""
    return np.clip(np.asarray(t, np.float32) * (2.0 ** s), -240.0, 240.0).astype(
        ml_dtypes.float8_e4m3
    )


def _prep_in_maps(x, Wu, bu, Wv, bv, Wg1, bg1, Wg2, bg2, conv_w, conv_b):
    bf = ml_dtypes.bfloat16
    f32 = np.float32
    x = np.asarray(x, f32)
    bu = np.asarray(bu, f32)
    bg1 = np.asarray(bg1, f32)
    bg2 = np.asarray(bg2, f32)
    zu = not bu.any()
    zg2 = not bg2.any()
    zbg1 = not bg1.any()
    WuT = np.asarray(Wu, f32).T
    WvT = np.asarray(Wv, f32).T
    # pre-shuffled DMA layouts: one contiguous run per SBUF partition.
    # WvPre[p, kg, rk, i*128+c] = WvT[kg*512 + i*128 + p, rk*128 + c]
    WvPre = np.ascontiguousarray(
        WvT.reshape(4, 4, P, NR, P).transpose(2, 0, 3, 1, 4).reshape(P, 4, NR, 4 * P)
    ).astype(bf)
    # Wu2Pre[p, rg, kd, i*128+c] = WuT[rg*512 + i*128 + p, kd*128 + c]
    Wu2Pre = np.ascontiguousarray(
        WuT.reshape(4, 4, P, ND, P).transpose(2, 0, 3, 1, 4).reshape(P, 4, ND, 4 * P)
    ).astype(bf)
    shared = dict(
        WuT=WuT.astype(bf),
        WvPre=WvPre,
        Wu2Pre=Wu2Pre,
        Wg18=np.ascontiguousarray(
            _q8(np.asarray(Wg1, f32).T, SW).reshape(ND, P, DH).transpose(1, 0, 2)
            .reshape(P, ND * DH)
        ),
        Wg28=np.ascontiguousarray(
            _q8(np.asarray(Wg2, f32).T, SW).reshape(NDH, P, R).transpose(1, 0, 2)
            .reshape(P, NDH * R)
        ),
        tri=(np.arange(P)[:, None] <= np.arange(P)[None, :]).astype(bf),
        bg1s_col=np.ascontiguousarray((bg1 * (2.0 ** SX)).reshape(NDH, P).T),
        bg2_col=np.ascontiguousarray(bg2.reshape(NR, P).T),
        bv_col=np.ascontiguousarray(np.asarray(bv, f32).reshape(NR, P).T),
        conv_w2=np.ascontiguousarray(
            np.asarray(conv_w, f32)[:, 0, :].reshape(ND, P, 3).transpose(1, 0, 2)
        ),
        conv_b2=np.ascontiguousarray(np.asarray(conv_b, f32).reshape(ND, P).T),
    )
    if not zu:
        shared["buB"] = np.broadcast_to(bu.astype(bf), (P, R)).copy()
    if not zg2:
        # pre-added into the psg PSUM (which sits at scale 2^(SX+SW))
        shared["bg2B"] = np.broadcast_to(
            (bg2 * (2.0 ** (SX + SW))).astype(bf), (P, R)
        ).copy()
    xflat = x.reshape(B * L, D)
    in_maps = []
    for c in range(NCORES):
        xh = np.zeros((TH, D), f32)
        xh[1 : T + 1] = xflat[c * T : (c + 1) * T]
        if c % 2 == 1:
            xh[0] = xflat[c * T - 1]
        else:
            xh[T + 1] = xflat[(c + 1) * T]
        odd = np.full((P, 1), float(c % 2), f32)
        in_maps.append(
            dict(
                shared,
                xT=xh.T.astype(bf),
                x8T=_q8(xflat[c * T : (c + 1) * T].T, SX),
                odd=odd,
            )
        )
    return in_maps, zu, zg2, zbg1


def _assemble(results):
    out = np.empty((B * L, D), np.float32)
    for c in range(NCORES):
        out[c * T : (c + 1) * T] = np.asarray(results[c]["outT"]).T
    return out.reshape(B, L, D)


def kernel(x, Wu, bu, Wv, bv, Wg1, bg1, Wg2, bg2, conv_w, conv_b):
    in_maps, zu, zg2, zbg1 = _prep_in_maps(
        x, Wu, bu, Wv, bv, Wg1, bg1, Wg2, bg2, conv_w, conv_b
    )
    res = run_bass_kernel_spmd(
        _get_nc(zu, zg2, zbg1), in_maps, core_ids=list(range(NCORES))
    )
    return _assemble(res.results)


def run_traced(inputs):
    """Profiled run: returns (output, exec_time_ns)."""
    in_maps, zu, zg2, zbg1 = _prep_in_maps(**inputs)
    res = run_bass_kernel_spmd(
        _get_nc(zu, zg2, zbg1), in_maps, core_ids=list(range(NCORES)), trace=True
    )
    return _assemble(res.results), res.exec_time_ns



# revision 32
# speedup vs baseline: 1.0151x; 1.0085x over previous
"""Trainium2 Bass kernel for AdaptiveSemiseparableLayer (v3).

v3: the whole gate MLP (H, G, and the transposed-gate recompute GT) runs
in fp8 e4m3 with MatmulPerfMode.DoubleRow — each matmul contracts 256
inputs in the time a bf16 matmul contracts 128, halving the PE cycles of
those phases (384 -> 192 matmul-equivalents). x/Wg1/Wg2 are quantized
host-side with static power-of-2 scales (x*2^5, W*2^10); the rescale
rides the existing evacuation ops (DVE tensor_scalar for relu, ACT
activation(scale=...) for the sigmoids). GT reuses the same quantized
h8/wg28 tiles as G, so gtt == gn^T exactly and the fp8 error is NOT paid
twice. Measured end-to-end rel err ~1.2e-2 (gate 2e-2). U/V/UV stay bf16:
fp8 there measures 3.7e-2..6.5e-2 — over the gate.

Reference computation (B=4, L=2048, D=R=2048, DH=512):
    t_out = depthwise_conv1d(x, conv_w, k=3) + conv_b
    u = x @ Wu.T + bu ; v = x @ Wv.T + bv
    gates = sigmoid(relu(x @ Wg1.T + bg1) @ Wg2.T + bg2)
    cs = cumsum(u * gates, axis=seq)
    out = t_out + (cs * (v * gates)) @ Wu.T

Sharding: sequence-parallel; 8192 tokens -> 8 contiguous 1024-token blocks
(one per core; each block lies inside one batch row). The only cross-core
dependency is the cumsum carry: odd core c needs core c-1's column sums.
That is a pairwise AllReduce(add) over (R,) = 8KB; each core then forms
carry = (pair_sum - own_sum) * is_odd on GPSIMD.

Changes vs the original kernel (637 us -> ~465 us):
  - two-level cumsum: 8 independent 128-wide triangular matmuls per r-tile
    (8 MMs, 1024 PE cycles) instead of the 12-MM trapezoid; the 8-block
    prefix is recombined per 128-token window on DVE via a scan +
    scalar_tensor_tensor (cs + P) * vg.
  - block sums come from the intra-cumsum PSUM's last columns (DVE strided
    gather + reduce) -- the 128 free-dim-1 block-sum matmuls are gone.
  - the carry selection matmuls (mask MMs) are gone (pairwise AllReduce
    over core pairs + (pair_sum - own_sum) * is_odd).
  - all bias matmuls are gone: bg1 rides a DVE tensor_scalar (add,max),
    bg2(GT) rides the ACT activation bias, bv rides a
    scalar_tensor_tensor, bu/bg2(G-natural) have a DVE broadcast-add
    fallback variant only when nonzero (they are zero here).
  - the tile scheduler reorders by dependency, not emission order, so all
    carry-dependent DVE work (and the x-only conv ct chains) is pinned via
    dummy 1-element WAW copies: the carry chain behind vgt(GATE), each
    kd's ct chain behind gos[kd]. Without these the DVE FIFO stalls on the
    ~35us AllReduce latency (or runs 32 ct chains before phase H's relus).
  - H is k-outer across all 8 PSUM banks so the 16 x chunks are consumed
    at DMA arrival pace; Wg1/Wg2 preloaded whole on the ACT DMA queue.
  - gn/intra and ug/gos share SBUF pools (phase-disjoint reuse); the
    final output tiles drain over three DMA queues.

Layouts (per core, T=1024 local tokens):
    xT    [D, T+2]  bf16   transposed shard with conv halo columns
    h     [dh-part, t]     (4 tiles)
    gn    [t-part, r]      (8 tiles)   natural gates (for ug)
    ug    [t-part, r]      bf16, lhsT of the intra cumsum matmuls
    intra [r-part, t]      bf16, block-local inclusive cumsum
    gtt/vgt [r-part, t]    transposed gates / gated v
    got   [r-part, t]      (cs_local + prefix + carry) * vg
    outT  [d-part, t]      final, conv fused in epilogue
"""

import numpy as np
import ml_dtypes
from contextlib import ExitStack

import concourse.bass as bass
import concourse.mybir as mybir
import concourse.tile as tile
from concourse.bass_utils import run_bass_kernel_spmd

P = 128
B, L, D = 4, 2048, 2048
R, DH = 2048, 512
NCORES = 8
T = (B * L) // NCORES          # 1024 tokens per core
TH = T + 2                     # with conv halo
ND, NR, NDH, NT = D // P, R // P, DH // P, T // P
TC = 512                       # matmul free-dim chunk (one PSUM bank of f32)
NTC = T // TC                  # 2
NRC = R // TC                  # 4
BF = mybir.dt.bfloat16
F8 = mybir.dt.float8e4
F32 = mybir.dt.float32
AF = mybir.ActivationFunctionType
ALU = mybir.AluOpType
DR = mybir.MatmulPerfMode.DoubleRow

# fp8 static power-of-2 scales for the gate MLP (x, Wg1/Wg2 quantized to
# TRN e4m3, max-normal 240): psh/psg land at 2^(SX+SW); the rescale rides
# the evacuation op (DVE mult / ACT activation scale).
SX = 5     # x, h are scaled by 2^SX
SW = 10    # Wg1, Wg2 scaled by 2^SW

import os
DG = int(os.environ.get("K_DG", "12"))    # got lag behind gtv emission


def _emit(nc, io, zu, zg2, zbg1):
    """zu/zg2/zbg1: True when bu / bg2 / bg1 are all-zero (skip bias work)."""
    ctx = ExitStack()
    tc = io["tc"]
    pool = lambda name, bufs, **kw: ctx.enter_context(
        tc.tile_pool(name=name, bufs=bufs, **kw)
    )
    const = pool("const", 1)
    xpool = pool("xpool", 1)
    hpool = pool("hpool", 1)       # single fp8 h tile [P, NDH, T]
    gnp = pool("gnp", NT)          # gn tiles; reused for intra pairs
    ugp = pool("ugp", 2 * NT)      # ug tiles [128,1024]; reused for gos
    pbp = pool("pbp", NR)          # Pb [128, 9] f32 per rk
    ptp = pool("ptp", 3)           # Ptot transient
    gtp = pool("gtp", 2)
    vgp = pool("vgp", DG + 2)
    wutp = pool("wutp", 6)
    wvtp = pool("wvtp", 8)
    wu2p = pool("wu2p", 5)
    outp = pool("outp", 2)
    ctp = pool("ctp", 4)
    psum = pool("psum", 8, space="PSUM")
    dram = pool("dram", 1, space="DRAM")

    xT, WuT = (io[k] for k in ["xT", "WuT"])
    WvPre, Wu2Pre = io["WvPre"], io["Wu2Pre"]
    x8T, Wg18, Wg28 = (io[k] for k in ["x8T", "Wg18", "Wg28"])
    outT = io["outT"]

    # ---- critical-path loads first: fp8 Wg1 + fp8 x shard (phase H inputs).
    # All weight/const layouts are pre-shuffled HOST-side so every DMA here
    # is one contiguous run per partition (~128 descriptors): the sequencer
    # DIRECT2D launch cost stays ~0.6us instead of 2-4us, which matters
    # because launches serialize on the issuing engine's queue (the ACT
    # queue must be clear before the phase-G sigmoids can run).
    wg18 = const.tile([P, ND, DH], F8)
    x8 = xpool.tile([P, ND, T], F8, name="x8", tag="x8")
    xtile = xpool.tile([P, ND, TH], BF, name="xt", tag="xt")
    # x8 even chunks on sync, odd on gpsimd: each H pair (2 chunks) arrives
    # at two-queue rate, ~2x the pair consumption rate of the DR matmuls.
    nc.scalar.dma_start(out=wg18[:, 0:2, :], in_=Wg18[:, 0 : 2 * DH])
    for k in (0, 1):
        eng = nc.sync if k % 2 == 0 else nc.gpsimd
        eng.dma_start(out=x8[:, k, :], in_=x8T[k * P : (k + 1) * P, :])
    if not zbg1:
        bg1sc = const.tile([P, NDH], F32)
        nc.scalar.dma_start(out=bg1sc, in_=io["bg1s_col"][:, :])
    nc.scalar.dma_start(out=wg18[:, 2:8, :], in_=Wg18[:, 2 * DH : 8 * DH])
    nc.scalar.dma_start(out=wg18[:, 8:16, :], in_=Wg18[:, 8 * DH : 16 * DH])
    # x8 evens on sync, odds 3..9 on gpsimd, odds 11..15 on scalar (behind
    # wg18): three streams keep every H pair well ahead of the PE. wg28
    # moves to gpsimd (needed only at phase G, ~23us in).
    for k in range(2, 10):
        eng = nc.sync if k % 2 == 0 else nc.gpsimd
        eng.dma_start(out=x8[:, k, :], in_=x8T[k * P : (k + 1) * P, :])
    wg28 = const.tile([P, NDH, R], F8)
    nc.gpsimd.dma_start(out=wg28, in_=Wg28[:, :])
    for k in range(10, ND):
        eng = nc.sync if k % 2 == 0 else nc.scalar
        eng.dma_start(out=x8[:, k, :], in_=x8T[k * P : (k + 1) * P, :])
    # PE warm-up: ~3.4us of dummy matmuls on memset data while the first x8
    # chunks stream in, so the HAM clock-gate reaches 8/8 (2.4 GHz) before
    # the first real matmul instead of ~15 matmuls into phase H.
    wrm = const.tile([P, 5 * P], BF)
    nc.vector.memset(wrm, 0.0)
    ps_wrm = psum.tile([P, TC], F32, name="ps_wrm", tag="ps")
    for i in range(16):
        nc.tensor.matmul(
            ps_wrm, lhsT=wrm[:, 0:P], rhs=wrm[:, P : P + TC],
            start=True, stop=True,
        )
    # bf16 x shard (needed from phase U on) behind x8 on the same two queues
    for k in range(ND):
        eng = nc.sync if k % 2 == 0 else nc.gpsimd
        eng.dma_start(out=xtile[:, k, :], in_=xT[k * P : (k + 1) * P, :])
    xs = [xtile[:, kd, :] for kd in range(ND)]

    # ---- remaining constants (all needed >=150us in), gpsimd queue
    tri = const.tile([P, P], BF)
    nc.gpsimd.dma_start(out=tri, in_=io["tri"][:, :])
    bg2c = const.tile([P, NR], F32)
    nc.gpsimd.dma_start(out=bg2c, in_=io["bg2_col"][:, :])
    bvc = const.tile([P, NR], F32)
    nc.gpsimd.dma_start(out=bvc, in_=io["bv_col"][:, :])
    cw = const.tile([P, ND, 3], F32)
    nc.gpsimd.dma_start(out=cw, in_=io["conv_w2"][:, :, :])
    cb = const.tile([P, ND], F32)
    nc.gpsimd.dma_start(out=cb, in_=io["conv_b2"][:, :])
    oddc = const.tile([P, 1], F32)
    nc.gpsimd.dma_start(out=oddc, in_=io["odd"][:, :])
    ones8 = const.tile([P, NT], F32)
    nc.vector.memset(ones8, 1.0)
    # pre-warm the ACT sigmoid table (1.3us ACT_TABLE_LOAD) during phase H
    # so the first real G sigmoid doesn't pay it on the PSUM-recycle path.
    warm = const.tile([P, 1], F32)
    nc.scalar.activation(out=warm[0:1, 0:1], in_=ones8[0:1, 0:1], func=AF.Sigmoid)
    if not zu:
        buB = const.tile([P, R], BF)
        nc.gpsimd.dma_start(out=buB, in_=io["buB"][:, :])
    if not zg2:
        bg2B = const.tile([P, R], BF)
        nc.gpsimd.dma_start(out=bg2B, in_=io["bg2B"][:, :])

    # ---- phase H (fp8 DoubleRow): h8 [dh-part, t] = relu(Wg1 @ x^T + bg1)*2^SX
    # pair-outer with all 8 PSUM banks; each DR matmul contracts two 128-d
    # chunks at bf16-matmul cost. psh lands at 2^(SX+SW); the 2^-SW rescale
    # rides the evacuation (DVE when bg1==0, ACT Relu bias/scale otherwise).
    h8 = hpool.tile([P, NDH, T], F8, name="h8", tag="h")
    psh = [[psum.tile([P, TC], F32, name=f"psh{kdh}{c}", tag="ps")
            for c in range(NTC)] for kdh in range(NDH)]
    for p in range(ND // 2):
        for kdh in range(NDH):
            for c in range(NTC):
                nc.tensor.matmul(
                    psh[kdh][c],
                    lhsT=wg18[:, 2 * p : 2 * p + 2, kdh * P : (kdh + 1) * P],
                    rhs=x8[:, 2 * p : 2 * p + 2, c * TC : (c + 1) * TC],
                    start=(p == 0),
                    stop=(p == ND // 2 - 1),
                    perf_mode=DR,
                )
    for kdh in range(NDH):
        for c in range(NTC):
            # evacuations split across DVE and ACT: phase G's first matmuls
            # reuse these PSUM banks, so halving the serial drain latency
            # starts G ~2us sooner.
            if zbg1 and c == 0:
                nc.vector.tensor_scalar(
                    h8[:, kdh, c * TC : (c + 1) * TC],
                    psh[kdh][c],
                    2.0 ** -SW,
                    0.0,
                    op0=ALU.mult,
                    op1=ALU.max,
                )
            else:
                nc.scalar.activation(
                    out=h8[:, kdh, c * TC : (c + 1) * TC], in_=psh[kdh][c],
                    func=AF.Relu,
                    bias=0.0 if zbg1 else bg1sc[:, kdh : kdh + 1],
                    scale=2.0 ** -SW,
                )

    # ---- phase G (fp8 DoubleRow): gn[t] [t-part, r] = sigmoid(h @ Wg2^T (+ bg2))
    gn = [gnp.tile([P, R], BF, name=f"gn{t}", tag="gn") for t in range(NT)]
    for rc in range(NRC):
        for h2 in range(2):
            psg = [psum.tile([P, TC], F32, name=f"psg{tt}", tag="ps")
                   for tt in range(NT // 2)]
            for p in range(NDH // 2):
                for tt in range(NT // 2):
                    t = h2 * (NT // 2) + tt
                    nc.tensor.matmul(
                        psg[tt],
                        lhsT=h8[:, 2 * p : 2 * p + 2, t * P : (t + 1) * P],
                        rhs=wg28[:, 2 * p : 2 * p + 2, rc * TC : (rc + 1) * TC],
                        start=(p == 0),
                        stop=(p == NDH // 2 - 1),
                        perf_mode=DR,
                    )
            for tt in range(NT // 2):
                t = h2 * (NT // 2) + tt
                if not zg2:
                    nc.vector.tensor_add(
                        out=psg[tt], in0=psg[tt],
                        in1=bg2B[:, rc * TC : (rc + 1) * TC],
                    )
                nc.scalar.activation(
                    out=gn[t][:, rc * TC : (rc + 1) * TC], in_=psg[tt],
                    func=AF.Sigmoid, scale=2.0 ** -(SX + SW),
                )

    # ---- phase U: ug[t] = (x @ Wu^T (+ bu)) * gn   [t-part, r]
    # ug tiles are [128, 1024]: two 512-wide rc chunks per tile.
    ugt = [[ugp.tile([P, 2 * TC], BF, name=f"ug_{t}_{rp}", tag="ug")
            for rp in range(2)] for t in range(NT)]
    bs_dram = dram.tile([P, NR], F32)
    for rc in range(NRC):
        psu = [psum.tile([P, TC], F32, name=f"psu{t}", tag="ps") for t in range(NT)]
        for k in range(ND):
            wut = wutp.tile([P, TC], BF, name="wut", tag="wut")
            nc.sync.dma_start(
                out=wut, in_=WuT[k * P : (k + 1) * P, rc * TC : (rc + 1) * TC]
            )
            for t in range(NT):
                nc.tensor.matmul(
                    psu[t],
                    lhsT=xs[k][:, 1 + t * P : 1 + t * P + P],
                    rhs=wut,
                    start=(k == 0),
                    stop=(k == ND - 1),
                )
        for t in range(NT):
            dst = ugt[t][rc // 2][:, (rc % 2) * TC : (rc % 2 + 1) * TC]
            if not zu:
                nc.vector.tensor_add(
                    out=psu[t], in0=psu[t],
                    in1=buB[:, rc * TC : (rc + 1) * TC],
                )
            nc.vector.tensor_mul(
                out=dst, in0=psu[t], in1=gn[t][:, rc * TC : (rc + 1) * TC],
            )

    # ---- phase CS (interleaved into GT/V below): block-local cumsum per
    # r-tile; extract block sums. intra[rk] lives in the gn pool (pairs).
    intra_t = [gnp.tile([P, R], BF, name=f"in{i}", tag="gn") for i in range(NT)]
    intra = [intra_t[rk // 2][:, (rk % 2) * T : (rk % 2 + 1) * T] for rk in range(NR)]
    bs_sb = const.tile([P, NR], F32)
    pbs = []
    deferred = []

    def flush_copy():
        # split the two fat PSUM->SBUF evacuations across ACT and DVE: the
        # PSUM pair recycles ~2x sooner, so the PE stalls less waiting for
        # free banks (ACT is ~9% busy here).
        frk, fps = deferred.pop(0)
        nc.scalar.activation(out=intra[frk][:, 0:TC], in_=fps[0], func=AF.Copy)
        nc.vector.tensor_copy(out=intra[frk][:, TC:T], in_=fps[1])

    def emit_cs(rk):
        rc4, ri4 = rk // 4, rk % 4
        ugsl = lambda j: ugt[j][rc4 // 2][:, (rc4 % 2) * TC + ri4 * P :
                                          (rc4 % 2) * TC + (ri4 + 1) * P]
        pscs = [psum.tile([P, TC], F32, name=f"pscs{c}", tag="ps") for c in range(2)]
        for j in range(NT):
            nc.tensor.matmul(
                pscs[j // 4][:, (j % 4) * P : (j % 4 + 1) * P],
                lhsT=ugsl(j),
                rhs=tri,
                start=True,
                stop=True,
            )
        # Pb gathers + block-sum reduce FIRST (they gate the collective);
        # the fat intra evacuation copies are deferred 1 r-tile.
        pb = pbp.tile([P, NT + 1], F32, name=f"pb{rk}", tag="pb")
        nc.vector.memset(pb[:, 0:1], 0.0)
        nc.vector.tensor_copy(out=pb[:, 1:5], in_=pscs[0][:, P - 1 :: P])
        nc.vector.tensor_copy(out=pb[:, 5:9], in_=pscs[1][:, P - 1 :: P])
        nc.vector.tensor_reduce(
            out=bs_sb[:, rk : rk + 1], in_=pb[:, 1:9],
            axis=mybir.AxisListType.X, op=ALU.add,
        )
        pbs.append(pb)
        deferred.append((rk, pscs))
        if len(deferred) > 1:
            flush_copy()

    pr_dram = dram.tile([P, NR], F32)
    prs = const.tile([P, NR], F32)
    carry = const.tile([P, NR], F32)

    def emit_allreduce():
        # [P, NR] end to end: contiguous per-partition DMAs (cheap launches),
        # and the elementwise AllReduce doesn't care about the layout. On the
        # gpsimd queue so its wait-for-bs_sb doesn't block the sync queue's
        # wvt loads (the V matmuls' feed).
        nc.gpsimd.dma_start(out=bs_dram[:, :], in_=bs_sb)
        nc.gpsimd.collective_compute(
            "AllReduce",
            ALU.add,
            replica_groups=[[2 * i, 2 * i + 1] for i in range(NCORES // 2)],
            ins=[bs_dram[:, :].opt()],
            outs=[pr_dram[:, :].opt()],
        )
        nc.gpsimd.dma_start(out=prs, in_=pr_dram[:, :])

    # ---- phase GT/V (PE+ACT+DVE) with got recombination on GPSIMD
    vgts = [None] * NR
    gos = [ugp.tile([P, T], BF, name=f"go{rk}", tag="ug") for rk in range(NR)]

    def emit_gtv(rk):
        # transposed gates, fp8 DR recompute from the SAME quantized h8/wg28
        # tiles as phase G: gtt == gn^T exactly (same products, same order).
        gtt = gtp.tile([P, T], BF, name="gtt", tag="gtt")
        psgt = [psum.tile([P, TC], F32, name=f"psgt{c}", tag="ps") for c in range(NTC)]
        for p in range(NDH // 2):
            for c in range(NTC):
                nc.tensor.matmul(
                    psgt[c],
                    lhsT=wg28[:, 2 * p : 2 * p + 2, rk * P : (rk + 1) * P],
                    rhs=h8[:, 2 * p : 2 * p + 2, c * TC : (c + 1) * TC],
                    start=(p == 0),
                    stop=(p == NDH // 2 - 1),
                    perf_mode=DR,
                )
        for c in range(NTC):
            nc.scalar.activation(
                out=gtt[:, c * TC : (c + 1) * TC],
                in_=psgt[c],
                func=AF.Sigmoid,
                bias=bg2c[:, rk : rk + 1],
                scale=2.0 ** -(SX + SW),
            )
        vgt = vgp.tile([P, T], BF, name="vgt", tag="vgt")
        psv = [psum.tile([P, TC], F32, name=f"psv{c}", tag="ps") for c in range(NTC)]
        for kg in range(ND // 4):
            wvt = wvtp.tile([P, 4 * P], BF, name="wvt", tag="wvt")
            nc.sync.dma_start(out=wvt, in_=WvPre[:, kg, rk, :])
            for i in range(4):
                k = kg * 4 + i
                for c in range(NTC):
                    nc.tensor.matmul(
                        psv[c],
                        lhsT=wvt[:, i * P : (i + 1) * P],
                        rhs=xs[k][:, 1 + c * TC : 1 + (c + 1) * TC],
                        start=(k == 0),
                        stop=(k == ND - 1),
                    )
        for c in range(NTC):
            nc.vector.scalar_tensor_tensor(
                out=vgt[:, c * TC : (c + 1) * TC],
                in0=psv[c],
                scalar=bvc[:, rk : rk + 1],
                in1=gtt[:, c * TC : (c + 1) * TC],
                op0=ALU.add,
                op1=ALU.mult,
            )
        return vgt

    GATE = int(os.environ.get("K_GATE", "6"))

    def emit_got(rk):
        if rk == 0:
            # The tile scheduler reorders by dependency, not emission order,
            # so a bare carry chain would be scheduled as soon as the DVE has
            # a gap -- stalling the whole DVE FIFO until the AllReduce lands.
            # This dummy 1-element copy makes the carry tile depend on
            # vgt(GATE), pinning the chain behind GATE r-tiles of GT/V work.
            nc.vector.tensor_copy(out=carry[0:1, 0:1], in_=vgts[GATE][0:1, 0:1])
            nc.vector.tensor_sub(out=carry, in0=prs, in1=bs_sb)
            nc.vector.tensor_scalar_mul(carry, carry, oddc[:, 0:1])
        # P_tot[:, j] = carry + sum_{j'<j} bsum_j'
        pt = ptp.tile([P, NT], F32, name="pt", tag="pt")
        nc.vector.tensor_tensor_scan(
            out=pt,
            data0=ones8,
            data1=pbs[rk][:, 0:NT],
            initial=carry[:, rk : rk + 1],
            op0=ALU.mult,
            op1=ALU.add,
        )
        for j in range(NT):
            nc.vector.scalar_tensor_tensor(
                out=gos[rk][:, j * P : (j + 1) * P],
                in0=intra[rk][:, j * P : (j + 1) * P],
                scalar=pt[:, j : j + 1],
                in1=vgts[rk][:, j * P : (j + 1) * P],
                op0=ALU.add,
                op1=ALU.mult,
            )
        vgts[rk] = None

    # CS r-tiles ride the first 4 GT/V iterations (4 per iteration): the
    # CS matmuls are tiny (N=128) and their PSUM drains are DVE/ACT-bound,
    # so standalone they leave the PE sparse AND let the HAM re-throttle
    # the clock; inside GT/V the PE stays dense and warm.
    for i in range(NR + DG):
        if i < NR:
            vgts[i] = emit_gtv(i)
            if i < 2:
                for rk in range(8 * i, 8 * i + 8):
                    emit_cs(rk)
                if i == 1:
                    while deferred:
                        flush_copy()
                    emit_allreduce()
        j = i - DG
        if j >= 0:
            emit_got(j)

    # ---- phase UV + conv epilogue: outT[d, t] = got-proj + conv + conv_b
    # The conv term ct depends only on x, so it is emitted BEFORE the kd's
    # matmuls: the DVE computes it while the PE accumulates, and the last
    # tile's epilogue is just one add + DMA after the final matmul.
    for kd in range(ND):
        wu2 = []
        for rg in range(NR // 4):
            wu24 = wu2p.tile([P, 4 * P], BF, name="wu24", tag="wu24")
            nc.sync.dma_start(out=wu24, in_=Wu2Pre[:, rg, kd, :])
            wu2.append(wu24)
        cts = []
        for c in range(NTC):
            ct = ctp.tile([P, TC], F32, name="ct", tag="ct")
            # dummy dep: ct chains depend only on x, and the scheduler would
            # otherwise run all 32 of them right after x lands -- ahead of
            # the phase-H relu evacuations, starving phase G. Gating each on
            # gos[kd] pins them into the UV phase where the DVE is idle.
            nc.vector.tensor_copy(out=ct[0:1, 0:1], in_=gos[kd][0:1, 0:1])
            nc.vector.tensor_scalar(
                ct,
                xs[kd][:, c * TC : c * TC + TC],
                cw[:, kd, 0:1],
                cb[:, kd : kd + 1],
                op0=ALU.mult,
                op1=ALU.add,
            )
            nc.vector.scalar_tensor_tensor(
                out=ct,
                in0=xs[kd][:, c * TC + 1 : c * TC + 1 + TC],
                scalar=cw[:, kd, 1:2],
                in1=ct,
                op0=ALU.mult,
                op1=ALU.add,
            )
            nc.vector.scalar_tensor_tensor(
                out=ct,
                in0=xs[kd][:, c * TC + 2 : c * TC + 2 + TC],
                scalar=cw[:, kd, 2:3],
                in1=ct,
                op0=ALU.mult,
                op1=ALU.add,
            )
            cts.append(ct)
        psuv = [psum.tile([P, TC], F32, name=f"psuv{c}", tag="ps") for c in range(NTC)]
        for rk in range(NR):
            for c in range(NTC):
                nc.tensor.matmul(
                    psuv[c],
                    lhsT=wu2[rk // 4][:, (rk % 4) * P : (rk % 4 + 1) * P],
                    rhs=gos[rk][:, c * TC : (c + 1) * TC],
                    start=(rk == 0),
                    stop=(rk == NR - 1),
                )
        for c in range(NTC):
            ob = outp.tile([P, TC], F32, name="ob", tag="ob")
            nc.vector.tensor_add(out=ob, in0=psuv[c], in1=cts[c])
            if kd >= ND - 2:
                # split the drain of the last tiles across four queues: the
                # final out DMA is the kernel's critical tail.
                for qi, eng in enumerate((nc.sync, nc.gpsimd, nc.scalar, nc.gpsimd)):
                    eng.dma_start(
                        out=outT[kd * P + qi * 32 : kd * P + (qi + 1) * 32,
                                 c * TC : (c + 1) * TC],
                        in_=ob[qi * 32 : (qi + 1) * 32, :],
                    )
            else:
                nc.sync.dma_start(
                    out=outT[kd * P : (kd + 1) * P, c * TC : (c + 1) * TC], in_=ob
                )
    ctx.close()


def _split_multi_waits(nc):
    """The walrus build in this env allows only ONE attached sync-wait per
    instruction; hoist extra waits onto standalone InstEventSemaphore ops
    inserted just before, on the same engine (semantically identical)."""
    import bass_rust

    n = 0
    for blk in nc.m.functions[0].blocks:
        changed = False
        out = []
        for ins in blk.instructions:
            si = getattr(ins, "sync_info", None)
            if si is not None and len(si.on_wait) > 1:
                waits = list(si.on_wait)
                for w in waits[:-1]:
                    ev = mybir.InstEventSemaphore(name=f"WSPLIT-{n}", ins=[], outs=[])
                    n += 1
                    ev.engine = ins.engine
                    ev.sync_info = bass_rust.SyncInfo(on_wait=[w], on_update=[])
                    out.append(ev)
                ins.sync_info = bass_rust.SyncInfo(
                    on_wait=[waits[-1]], on_update=list(si.on_update)
                )
                changed = True
            out.append(ins)
        if changed:
            try:
                blk.instructions[:] = out
            except TypeError:
                blk.instructions = out
    return n


def _build(zu, zg2, zbg1):
    nc = bass.Bass(num_devices=NCORES)
    io = {}
    io["xT"] = nc.declare_dram_parameter("xT", [D, TH], BF, False)
    io["x8T"] = nc.declare_dram_parameter("x8T", [D, T], F8, False)
    io["WuT"] = nc.declare_dram_parameter("WuT", [D, R], BF, False)
    io["WvPre"] = nc.declare_dram_parameter("WvPre", [P, 4, NR, 4 * P], BF, False)
    io["Wu2Pre"] = nc.declare_dram_parameter("Wu2Pre", [P, NR // 4, ND, 4 * P], BF, False)
    io["Wg18"] = nc.declare_dram_parameter("Wg18", [P, ND * DH], F8, False)
    io["Wg28"] = nc.declare_dram_parameter("Wg28", [P, NDH * R], F8, False)
    io["tri"] = nc.declare_dram_parameter("tri", [P, P], BF, False)
    io["bg1s_col"] = nc.declare_dram_parameter("bg1s_col", [P, NDH], F32, False)
    io["bg2_col"] = nc.declare_dram_parameter("bg2_col", [P, NR], F32, False)
    io["bv_col"] = nc.declare_dram_parameter("bv_col", [P, NR], F32, False)
    io["conv_w2"] = nc.declare_dram_parameter("conv_w2", [P, ND, 3], F32, False)
    io["conv_b2"] = nc.declare_dram_parameter("conv_b2", [P, ND], F32, False)
    io["odd"] = nc.declare_dram_parameter("odd", [P, 1], F32, False)
    if not zu:
        io["buB"] = nc.declare_dram_parameter("buB", [P, R], BF, False)
    if not zg2:
        io["bg2B"] = nc.declare_dram_parameter("bg2B", [P, R], BF, False)
    io["outT"] = nc.declare_dram_parameter("outT", [D, T], F32, True)
    with tile.TileContext(nc, num_cores=NCORES) as tc:
        io["tc"] = tc
        _emit(nc, io, zu, zg2, zbg1)
    _split_multi_waits(nc)
    return nc


_NC_CACHE = {}


def _get_nc(zu, zg2, zbg1):
    key = (zu, zg2, zbg1)
    if key not in _NC_CACHE:
        _NC_CACHE[key] = _build(zu, zg2, zbg1)
    return _NC_CACHE[key]


def _q8(t, s):
    """TRN e4m3 quantization with power-of-2 scale 2^s (clip to max normal)."""
    return np.clip(np.asarray(t, np.float32) * (2.0 ** s), -240.0, 240.0).astype(
        ml_dtypes.float8_e4m3
    )


def _prep_in_maps(x, Wu, bu, Wv, bv, Wg1, bg1, Wg2, bg2, conv_w, conv_b):
    bf = ml_dtypes.bfloat16
    f32 = np.float32
    x = np.asarray(x, f32)
    bu = np.asarray(bu, f32)
    bg1 = np.asarray(bg1, f32)
    bg2 = np.asarray(bg2, f32)
    zu = not bu.any()
    zg2 = not bg2.any()
    zbg1 = not bg1.any()
    WuT = np.asarray(Wu, f32).T
    WvT = np.asarray(Wv, f32).T
    # pre-shuffled DMA layouts: one contiguous run per SBUF partition.
    # WvPre[p, kg, rk, i*128+c] = WvT[kg*512 + i*128 + p, rk*128 + c]
    WvPre = np.ascontiguousarray(
        WvT.reshape(4, 4, P, NR, P).transpose(2, 0, 3, 1, 4).reshape(P, 4, NR, 4 * P)
    ).astype(bf)
    # Wu2Pre[p, rg, kd, i*128+c] = WuT[rg*512 + i*128 + p, kd*128 + c]
    Wu2Pre = np.ascontiguousarray(
        WuT.reshape(4, 4, P, ND, P).transpose(2, 0, 3, 1, 4).reshape(P, 4, ND, 4 * P)
    ).astype(bf)
    shared = dict(
        WuT=WuT.astype(bf),
        WvPre=WvPre,
        Wu2Pre=Wu2Pre,
        Wg18=np.ascontiguousarray(
            _q8(np.asarray(Wg1, f32).T, SW).reshape(ND, P, DH).transpose(1, 0, 2)
            .reshape(P, ND * DH)
        ),
        Wg28=np.ascontiguousarray(
            _q8(np.asarray(Wg2, f32).T, SW).reshape(NDH, P, R).transpose(1, 0, 2)
            .reshape(P, NDH * R)
        ),
        tri=(np.arange(P)[:, None] <= np.arange(P)[None, :]).astype(bf),
        bg1s_col=np.ascontiguousarray((bg1 * (2.0 ** SX)).reshape(NDH, P).T),
        bg2_col=np.ascontiguousarray(bg2.reshape(NR, P).T),
        bv_col=np.ascontiguousarray(np.asarray(bv, f32).reshape(NR, P).T),
        conv_w2=np.ascontiguousarray(
            np.asarray(conv_w, f32)[:, 0, :].reshape(ND, P, 3).transpose(1, 0, 2)
        ),
        conv_b2=np.ascontiguousarray(np.asarray(conv_b, f32).reshape(ND, P).T),
    )
    if not zu:
        shared["buB"] = np.broadcast_to(bu.astype(bf), (P, R)).copy()
    if not zg2:
        # pre-added into the psg PSUM (which sits at scale 2^(SX+SW))
        shared["bg2B"] = np.broadcast_to(
            (bg2 * (2.0 ** (SX + SW))).astype(bf), (P, R)
        ).copy()
    xflat = x.reshape(B * L, D)
    in_maps = []
    for c in range(NCORES):
        xh = np.zeros((TH, D), f32)
        xh[1 : T + 1] = xflat[c * T : (c + 1) * T]
        if c % 2 == 1:
            xh[0] = xflat[c * T - 1]
        else:
            xh[T + 1] = xflat[(c + 1) * T]
        odd = np.full((P, 1), float(c % 2), f32)
        in_maps.append(
            dict(
                shared,
                xT=xh.T.astype(bf),
                x8T=_q8(xflat[c * T : (c + 1) * T].T, SX),
                odd=odd,
            )
        )
    return in_maps, zu, zg2, zbg1


def _assemble(results):
    out = np.empty((B * L, D), np.float32)
    for c in range(NCORES):
        out[c * T : (c + 1) * T] = np.asarray(results[c]["outT"]).T
    return out.reshape(B, L, D)


def kernel(x, Wu, bu, Wv, bv, Wg1, bg1, Wg2, bg2, conv_w, conv_b):
    in_maps, zu, zg2, zbg1 = _prep_in_maps(
        x, Wu, bu, Wv, bv, Wg1, bg1, Wg2, bg2, conv_w, conv_b
    )
    res = run_bass_kernel_spmd(
        _get_nc(zu, zg2, zbg1), in_maps, core_ids=list(range(NCORES))
    )
    return _assemble(res.results)


def run_traced(inputs):
    """Profiled run: returns (output, exec_time_ns)."""
    in_maps, zu, zg2, zbg1 = _prep_in_maps(**inputs)
    res = run_bass_kernel_spmd(
        _get_nc(zu, zg2, zbg1), in_maps, core_ids=list(range(NCORES)), trace=True
    )
    return _assemble(res.results), res.exec_time_ns

